# revision 1
# baseline (speedup 1.0000x reference)
"""Bass/Tile TRN2 kernel for nn_DimensionScaledEqProp.

Data-parallel over batch: x rows sharded across 8 NeuronCores, weights
replicated. Per-core state (h) stays resident in SBUF across the 30
sequential steps. fp16 matmul operands, fp32 accumulation/state.

Self-contained: hardcodes shapes; host side does sharding, spectral-norm
sigma (tiny: 60 matvecs), weight folding/transposition, and the final
gather/transpose.
"""
import sys
import numpy as np

for _p in ("/opt/trn_rl_repo", "/root/.axon_site/_ro/trn_rl_repo"):
    if _p not in sys.path:
        sys.path.append(_p)

B, DIN, DH, DOUT = 4096, 512, 1024, 256
DFF = 4 * DH
STEPS = 30
N_CORES = 8
R = B // N_CORES  # rows per core = 512
GAMMA = 0.5 * min(1.0, float(np.sqrt(64.0 / DIN)))
LN_EPS = 1e-5

KD = DH // 128    # 8  k-tiles over DH
FD = DFF // 128   # 32 f-tiles over DFF
RD = R // 128     # 4  row-tiles per core
ID = DIN // 128   # 4  k-tiles over DIN
OD = DOUT // 128  # 2  out-tiles over DOUT
NH = DH // 512    # 2  psum halves over DH

_CACHE = {}


def _build_program(steps: int):
    import concourse.bass as bass
    import concourse.bacc as bacc
    import concourse.mybir as mybir
    from concourse import tile, masks

    f16 = mybir.dt.float16
    f32 = mybir.dt.float32
    AF = mybir.ActivationFunctionType
    OP = mybir.AluOpType

    nc = bacc.Bacc("TRN2", target_bir_lowering=False, debug=False,
                   enable_asserts=True, num_devices=N_CORES)

    xT_d = nc.dram_tensor("xT", [DIN, R], f16, kind="ExternalInput")
    ewT_d = nc.dram_tensor("ewT", [DIN, DH], f16, kind="ExternalInput")
    w1t_d = nc.dram_tensor("w1t", [DH, DFF], f16, kind="ExternalInput")
    b1e_d = nc.dram_tensor("b1e", [DFF, 1], f32, kind="ExternalInput")
    w2t_d = nc.dram_tensor("w2t", [DFF, DH], f16, kind="ExternalInput")
    vb1_d = nc.dram_tensor("vb1", [1, DH], f32, kind="ExternalInput")
    vb2_d = nc.dram_tensor("vb2", [1, DH], f32, kind="ExternalInput")
    hwt_d = nc.dram_tensor("hwt", [DH, DOUT], f16, kind="ExternalInput")
    hb_d = nc.dram_tensor("hb", [DOUT, 1], f32, kind="ExternalInput")
    outT_d = nc.dram_tensor("outT", [DOUT, R], f32, kind="ExternalOutput")

    with tile.TileContext(nc) as tc:
        with (
            tc.tile_pool(name="wp", bufs=1) as wp,
            tc.tile_pool(name="sp", bufs=1) as sp,
            tc.tile_pool(name="wk", bufs=2) as wk,
            tc.tile_pool(name="stp", bufs=2) as stp,
            tc.tile_pool(name="pst", bufs=4, space="PSUM") as pst,
            tc.tile_pool(name="ps1", bufs=2, space="PSUM") as ps1p,
            tc.tile_pool(name="ps2", bufs=2, space="PSUM") as ps2p,
        ):
            # ---- persistent weights / constants ----
            w1 = [wp.tile([128, DFF], f16, name=f"w1_{k}") for k in range(KD)]
            w2 = [wp.tile([128, DH], f16, name=f"w2_{f}") for f in range(FD)]
            hwt = [wp.tile([128, DOUT], f16, name=f"hwt_{k}") for k in range(KD)]
            b1s = wp.tile([128, FD], f32, name="b1s")
            hbs = wp.tile([128, OD], f32, name="hbs")
            ident = wp.tile([128, 128], f16, name="ident")

            # ---- persistent state ----
            h = [sp.tile([128, DH], f32, name=f"h_{r}") for r in range(RD)]
            xeg = [sp.tile([128, DH], f16, name=f"xeg_{r}") for r in range(RD)]
            hnT = [sp.tile([128, R], f16, name=f"hnT_{k}") for k in range(KD)]

            masks.make_identity(nc, ident[:])

            # ---- embed (transient pool, released before the step loop) ----
            with tc.tile_pool(name="ep", bufs=1) as ep:
                xts = [ep.tile([128, R], f16, name=f"xts_{i}")
                       for i in range(ID)]
                ewt = [ep.tile([128, DH], f16, name=f"ewt_{i}")
                       for i in range(ID)]
                bc1 = ep.tile([128, DH], f32, name="bc1")
                bc2 = ep.tile([128, DH], f32, name="bc2")
                for i in range(ID):
                    nc.sync.dma_start(
                        xts[i][:], xT_d.ap()[i * 128:(i + 1) * 128, :])
                    nc.sync.dma_start(
                        ewt[i][:], ewT_d.ap()[i * 128:(i + 1) * 128, :])
                nc.sync.dma_start(bc1[0:1, :], vb1_d.ap())
                nc.sync.dma_start(bc2[0:1, :], vb2_d.ap())
                nc.gpsimd.partition_broadcast(bc1[:], bc1[0:1, :])
                nc.gpsimd.partition_broadcast(bc2[:], bc2[0:1, :])

                # weight loads AFTER embed inputs: embed matmuls start
                # immediately; w1/w2 stream in behind them
                for k in range(KD):
                    nc.sync.dma_start(
                        w1[k][:], w1t_d.ap()[k * 128:(k + 1) * 128, :])
                nc.sync.dma_start(
                    b1s[:], b1e_d.ap().rearrange("(f p) o -> p (f o)", p=128))
                nc.sync.dma_start(
                    hbs[:], hb_d.ap().rearrange("(t p) o -> p (t o)", p=128))

                # h0 = x @ ewT + embed_b ; xeg = g*h0 + g*b2 (f16)
                for r in range(RD):
                    for half in range(NH):
                        sl = slice(half * 512, (half + 1) * 512)
                        pe = ps1p.tile([128, 512], f32, tag="ps1", name="pe")
                        for i in range(ID):
                            nc.tensor.matmul(
                                pe[:], xts[i][:, r * 128:(r + 1) * 128],
                                ewt[i][:, sl],
                                start=(i == 0), stop=(i == ID - 1))
                        nc.vector.tensor_tensor(
                            h[r][:, sl], pe[:], bc1[:, sl], op=OP.add)
                        nc.vector.scalar_tensor_tensor(
                            xeg[r][:, sl], h[r][:, sl], GAMMA, bc2[:, sl],
                            op0=OP.mult, op1=OP.add)

            # ---- initial LN stats on h0 (ACT sqrt once; hides table load) ----
            mv0 = stp.tile([128, RD * 2], f32, tag="mv", name="mv_init")
            for r in range(RD):
                st6 = stp.tile([128, 12], f32, tag="st6", name=f"st6_i_{r}")
                for c in range(2):
                    nc.vector.bn_stats(
                        st6[:, c * 6:(c + 1) * 6],
                        h[r][:, c * 512:(c + 1) * 512])
                nc.vector.bn_aggr(
                    mv0[:].rearrange("p (r x) -> p r x", x=2)[:, r], st6[:])
            mvv0 = mv0[:].rearrange("p (r x) -> p r x", x=2)
            ve0 = stp.tile([128, RD], f32, tag="ve", name="ve_init")
            nc.vector.tensor_scalar(
                ve0[:], mv0[:].rearrange("p (r x) -> p x r", x=2)[:, 1], LN_EPS, None, op0=OP.add)
            rv0 = stp.tile([128, RD], f32, tag="rv", name="rv_init")
            nc.vector.reciprocal(rv0[:], ve0[:])
            rs_prev = stp.tile([128, RD], f32, tag="rs", name="rs_init")
            nc.scalar.activation(rs_prev[:], rv0[:], AF.Sqrt)

            # ---- hidT pool reuses the embed pool space ----
            with tc.tile_pool(name="hp", bufs=1) as hp:
                hidT = [hp.tile([128, R], f16, name=f"hidT_{f}")
                        for f in range(FD)]

                def rstd_newton(y_out, y_seed, var_ap, tag_sfx, n_iter=2):
                    """y_out[128,1] = 1/sqrt(var+eps) via Newton from seed."""
                    hv = stp.tile([128, 1], f32, tag="hv",
                                  name=f"hv_{tag_sfx}")
                    nc.vector.tensor_scalar(
                        hv[:], var_ap, -0.5, -0.5 * LN_EPS,
                        op0=OP.mult, op1=OP.add)
                    y = y_seed
                    for it in range(n_iter):
                        a = stp.tile([128, 1], f32, tag="nwa",
                                     name=f"nwa_{tag_sfx}_{it}")
                        nc.vector.tensor_tensor(a[:], y, y, op=OP.mult)
                        nc.vector.tensor_scalar(
                            a[:], a[:], hv[:], 1.5, op0=OP.mult, op1=OP.add)
                        if it == n_iter - 1:
                            nc.vector.tensor_tensor(y_out, y, a[:], op=OP.mult)
                        else:
                            yn = stp.tile([128, 1], f32, tag="nwy",
                                          name=f"nwy_{tag_sfx}_{it}")
                            nc.vector.tensor_tensor(yn[:], y, a[:], op=OP.mult)
                            y = yn[:]

                # normalize h0 -> hnT for step 0 (one Newton polish on seed)
                mv_p, rs_p = mv0, rs_prev
                rs_fix = stp.tile([128, RD], f32, tag="rsf", name="rs_fix")
                for r in range(RD):
                    rstd_newton(rs_fix[:, r:r + 1], rs_prev[:, r:r + 1],
                                mv0[:, 2 * r + 1:2 * r + 2], f"i{r}", n_iter=1)
                rs_p = rs_fix

                def normalize(r, mean_ap, rs_col, sfx):
                    nmu = stp.tile([128, 1], f32, tag="nmu", name=f"nmu_{sfx}")
                    nc.vector.scalar_tensor_tensor(
                        nmu[:], mean_ap, -1.0, rs_col,
                        op0=OP.mult, op1=OP.mult)
                    hn16 = wk.tile([128, DH], f16, tag=f"hn16_{r}",
                                   name=f"hn16_{sfx}", bufs=1)
                    # two half-width ops so transposes of the low half can
                    # start before the high half is normalized
                    for half in range(NH):
                        sl = slice(half * 512, (half + 1) * 512)
                        nc.vector.tensor_scalar(
                            hn16[:, sl], h[r][:, sl], rs_col, nmu[:],
                            op0=OP.mult, op1=OP.add)
                    return hn16

                def transposes(r, hn16, sfx):
                    for k in range(KD):
                        tp = pst.tile([128, 128], f16, tag="tp",
                                      name=f"tp_{sfx}_{k}")
                        nc.tensor.transpose(
                            tp[:], hn16[:, k * 128:(k + 1) * 128], ident[:])
                        if k % 2 == 0:
                            nc.vector.tensor_copy(
                                hnT[k][:, r * 128:(r + 1) * 128], tp[:])
                        else:
                            nc.scalar.copy(
                                hnT[k][:, r * 128:(r + 1) * 128], tp[:])

                for r in range(RD):
                    hn = normalize(r, mv0[:, 2 * r:2 * r + 1],
                                   rs_p[:, r:r + 1], f"s0_{r}")
                    transposes(r, hn, f"s0_{r}")

                for s in range(steps):
                    last = (s == steps - 1)
                    # hidT = tanh(W1n' @ hnT + b1)
                    for f in range(FD):
                        p1 = ps1p.tile([128, 512], f32, tag="ps1",
                                       name=f"p1_{s}_{f}")
                        for k in range(KD):
                            nc.tensor.matmul(
                                p1[:], w1[k][:, f * 128:(f + 1) * 128],
                                hnT[k][:],
                                start=(k == 0), stop=(k == KD - 1))
                        nc.scalar.activation(
                            hidT[f][:], p1[:], AF.Tanh, bias=b1s[:, f:f + 1])

                    if s == 0:
                        # w2/hwt loads deferred past step-0 mm1 so w1 gets
                        # full DMA bandwidth at startup; w2 arrives during
                        # mm1 execution, well before mm2 needs it
                        for f_ in range(FD):
                            nc.sync.dma_start(
                                w2[f_][:],
                                w2t_d.ap()[f_ * 128:(f_ + 1) * 128, :])
                        for k_ in range(KD):
                            nc.sync.dma_start(
                                hwt[k_][:],
                                hwt_d.ap()[k_ * 128:(k_ + 1) * 128, :])

                    # per row-tile: matmul2 (+xeg seeded in PSUM), update,
                    # stats, rstd, normalize, transpose -- interleaved so PE
                    # never idles at the step boundary.
                    mv = stp.tile([128, RD * 2], f32, tag="mv",
                                  name=f"mv_{s}")
                    mvv = mv[:].rearrange("p (r x) -> p r x", x=2)
                    rs = stp.tile([128, RD], f32, tag="rs", name=f"rs_{s}")
                    hns = {}
                    for r in range(RD):
                        st6 = stp.tile([128, 12], f32, tag="st6",
                                       name=f"st6_{s}_{r}")
                        for half in range(NH):
                            sl = slice(half * 512, (half + 1) * 512)
                            p2 = ps2p.tile([128, 512], f32, tag="ps2",
                                           name=f"p2_{s}_{r}_{half}")
                            nc.tensor.matmul(
                                p2[:], ident[:], xeg[r][:, sl],
                                start=True, stop=False)
                            for f in range(FD):
                                nc.tensor.matmul(
                                    p2[:], hidT[f][:, r * 128:(r + 1) * 128],
                                    w2[f][:, sl],
                                    start=False, stop=(f == FD - 1))
                            nc.vector.scalar_tensor_tensor(
                                h[r][:, sl], h[r][:, sl], 1.0 - GAMMA, p2[:],
                                op0=OP.mult, op1=OP.add)
                            if not last:
                                # stats chunk for this half right away
                                nc.vector.bn_stats(
                                    st6[:, half * 6:(half + 1) * 6],
                                    h[r][:, sl])
                        if last:
                            # head prep inline: cast final h to fp16 so its
                            # transposes overlap the remaining matmul2 groups
                            hc16 = wk.tile([128, DH], f16, tag=f"hn16_{r}",
                                           name=f"hc16_{r}", bufs=1)
                            nc.vector.tensor_copy(hc16[:], h[r][:])
                            hns[r] = hc16
                            continue
                        nc.vector.bn_aggr(mvv[:, r], st6[:])
                        rstd_newton(rs[:, r:r + 1], rs_p[:, r:r + 1],
                                    mv[:, 2 * r + 1:2 * r + 2], f"{s}_{r}")
                        hns[r] = normalize(r, mv[:, 2 * r:2 * r + 1],
                                           rs[:, r:r + 1], f"{s}_{r}")
                    # transposes LAST: PE has cover work while the final
                    # row-tile's DVE chain drains, so it never idles
                    for r in range(RD):
                        transposes(r, hns[r], f"{s}_{r}")
                    mv_p, rs_p = mv, rs

                # ---- head: outT = head_w @ h.T + head_b ----
                # (hnT already holds final h transposed, prepped in-loop)
                for ot in range(OD):
                    po = ps1p.tile([128, 512], f32, tag="ps1", name=f"po_{ot}")
                    for k in range(KD):
                        nc.tensor.matmul(
                            po[:], hwt[k][:, ot * 128:(ot + 1) * 128],
                            hnT[k][:],
                            start=(k == 0), stop=(k == KD - 1))
                    osb = wk.tile([128, 512], f32, tag="osb",
                                  name=f"osb_{ot}", bufs=1)
                    nc.scalar.activation(
                        osb[:], po[:], AF.Identity, bias=hbs[:, ot:ot + 1])
                    nc.sync.dma_start(
                        outT_d.ap()[ot * 128:(ot + 1) * 128, :], osb[:])

    nc.compile()
    return nc


def _get_compiled(steps: int):
    key = ("prog", steps)
    if key not in _CACHE:
        from concourse.bass_interp import get_hw_module
        nc = _build_program(steps)
        nc.m = get_hw_module(nc.m)
        _CACHE[key] = nc
    return _CACHE[key]


def _spectral_sigma(W: np.ndarray) -> float:
    W = W.astype(np.float64)
    v = np.full(W.shape[1], 1.0 / np.sqrt(W.shape[1]))
    u = W @ v
    u = u / (np.linalg.norm(u) + 1e-12)
    for _ in range(15):
        u = W @ v
        u = u / (np.linalg.norm(u) + 1e-12)
        v = W.T @ u
        v = v / (np.linalg.norm(v) + 1e-12)
    return float(u @ (W @ v))


def _prep_host(inputs: dict) -> tuple[dict, list]:
    f = {k: np.asarray(v, dtype=np.float32) for k, v in inputs.items()}
    x, ew, eb = f["x"], f["embed_w"], f["embed_b"]
    W1, b1, W2, b2 = f["W1"], f["b1"], f["W2"], f["b2"]
    ln_g, ln_b = f["ln_g"], f["ln_b"]
    hw_, hb = f["head_w"], f["head_b"]

    s1 = _spectral_sigma(W1)
    s2 = _spectral_sigma(W2)
    W1n = (W1.astype(np.float64) / s1)
    W2n = (W2.astype(np.float64) / s2)
    # fold ln gain into W1, ln bias into b1
    W1eff = W1n * ln_g.astype(np.float64)[None, :]
    b1eff = (b1.astype(np.float64) + W1n @ ln_b.astype(np.float64))
    W2eff = GAMMA * W2n

    shared = {
        "ewT": np.ascontiguousarray(ew.T).astype(np.float16),
        "w1t": np.ascontiguousarray(W1eff.T).astype(np.float16),
        "b1e": b1eff.astype(np.float32).reshape(DFF, 1),
        "w2t": np.ascontiguousarray(W2eff.T).astype(np.float16),
        "vb1": eb.reshape(1, DH).astype(np.float32),
        "vb2": (GAMMA * b2).reshape(1, DH).astype(np.float32),
        "hwt": np.ascontiguousarray(hw_.T).astype(np.float16),
        "hb": hb.reshape(DOUT, 1).astype(np.float32),
    }
    in_maps = []
    for c in range(N_CORES):
        shard = x[c * R:(c + 1) * R, :]
        m = dict(shared)
        m["xT"] = np.ascontiguousarray(shard.T).astype(np.float16)
        in_maps.append(m)
    return shared, in_maps


def kernel(**inputs) -> np.ndarray:
    from concourse import bass_utils
    nc = _get_compiled(STEPS)
    _, in_maps = _prep_host(inputs)
    res = None
    for attempt in range(3):
        try:
            res = bass_utils.run_bass_kernel_spmd(
                nc, in_maps, core_ids=list(range(N_CORES)))
            break
        except Exception:
            # transient NRT_EXEC_UNIT_UNRECOVERABLE device wedges clear on
            # retry
            if attempt == 2:
                raise
    out = np.empty((B, DOUT), np.float32)
    for c in range(N_CORES):
        out[c * R:(c + 1) * R, :] = res.results[c]["outT"].T
    return out


if __name__ == "__main__":
    rng = np.random.default_rng(0)
    demo = {
        "x": rng.standard_normal((B, DIN)).astype(np.float32),
        "embed_w": (rng.standard_normal((DH, DIN)) * 0.02).astype(np.float32),
        "embed_b": np.zeros(DH, np.float32),
        "W1": (rng.standard_normal((DFF, DH)) * 0.02).astype(np.float32),
        "b1": np.zeros(DFF, np.float32),
        "W2": (rng.standard_normal((DH, DFF)) * 0.02).astype(np.float32),
        "b2": np.zeros(DH, np.float32),
        "ln_g": np.ones(DH, np.float32),
        "ln_b": np.zeros(DH, np.float32),
        "head_w": (rng.standard_normal((DOUT, DH)) * 0.02).astype(np.float32),
        "head_b": np.zeros(DOUT, np.float32),
    }
    out = kernel(**demo)
    print("out", out.shape, out.dtype, float(np.abs(out).max()))



# revision 3
# speedup vs baseline: 1.4517x; 1.4517x over previous
"""Bass/Tile TRN2 kernel for nn_DimensionScaledEqProp.

Data-parallel over batch: x rows sharded across 8 NeuronCores, weights
replicated. Per-core state (h) stays resident in SBUF across the 30
sequential steps.

fp8 (e4m3) DoubleRow matmuls for the two big per-step GEMMs (2x PE
throughput vs fp16). Numerics plan:
  - h state carried scaled by 2^8 (H = 256*h) so W2eff*256 quantizes to
    fp8 in a good exponent range; LN is scale-invariant so hn == LN(h).
  - W1eff scaled by 2^6 into fp8; descale folded into ACT tanh scale.
  - hn/hid quantized to fp8 on the fly (cast copies / ACT output).
  - PSUM accumulation fp32; h state fp32; LN/update path fp32.
  - head matmul fp16 with head_w/16 and ACT post-scale 2^-4/2^-8.
CPU-simulated rel err 1.4e-2 (gate 2e-2).

Self-contained: hardcodes shapes; host side does sharding, spectral-norm
sigma (tiny: 60 matvecs), weight folding/quantization, and the final
gather/transpose.
"""
import sys
import numpy as np

for _p in ("/opt/trn_rl_repo", "/root/.axon_site/_ro/trn_rl_repo"):
    if _p not in sys.path:
        sys.path.append(_p)

B, DIN, DH, DOUT = 4096, 512, 1024, 256
DFF = 4 * DH
STEPS = 30
N_CORES = 8
R = B // N_CORES  # rows per core = 512
GAMMA = 0.5 * min(1.0, float(np.sqrt(64.0 / DIN)))
LN_EPS = 1e-5

KD = DH // 128    # 8  k-tiles over DH
FD = DFF // 128   # 32 f-tiles over DFF
RD = R // 128     # 4  row-tiles per core
ID = DIN // 128   # 4  k-tiles over DIN
OD = DOUT // 128  # 2  out-tiles over DOUT
NH = DH // 512    # 2  psum halves over DH
KP = KD // 2      # 4  k-pairs over DH (DoubleRow)
FP = FD // 2      # 16 f-pairs over DFF (DoubleRow)

ASC = 256.0       # h-state scale 2^8
WS = 64.0         # W1 fp8 scale 2^6
HSC = 16.0        # head weight scale 2^4

_CACHE = {}


def _build_program(steps: int):
    import concourse.bass as bass
    import concourse.bacc as bacc
    import concourse.mybir as mybir
    from concourse import tile, masks

    f8 = mybir.dt.float8e4
    f16 = mybir.dt.float16
    f32 = mybir.dt.float32
    AF = mybir.ActivationFunctionType
    OP = mybir.AluOpType
    DR = mybir.MatmulPerfMode.DoubleRow

    nc = bacc.Bacc("TRN2", target_bir_lowering=False, debug=False,
                   enable_asserts=True, num_devices=N_CORES)

    xT_d = nc.dram_tensor("xT", [DIN, R], f16, kind="ExternalInput")
    ewT_d = nc.dram_tensor("ewT", [DIN, DH], f16, kind="ExternalInput")
    w1q_d = nc.dram_tensor("w1q", [KP * 128, 2 * DFF], f8,
                           kind="ExternalInput")
    b1e_d = nc.dram_tensor("b1e", [DFF, 1], f32, kind="ExternalInput")
    w2q_d = nc.dram_tensor("w2q", [FP * 128, 2 * DH], f8,
                           kind="ExternalInput")
    vb1_d = nc.dram_tensor("vb1", [1, DH], f32, kind="ExternalInput")
    vb2_d = nc.dram_tensor("vb2", [1, DH], f32, kind="ExternalInput")
    hwt_d = nc.dram_tensor("hwt", [DH, DOUT], f16, kind="ExternalInput")
    hb_d = nc.dram_tensor("hb", [DOUT, 1], f32, kind="ExternalInput")
    outT_d = nc.dram_tensor("outT", [DOUT, R], f32, kind="ExternalOutput")

    with tile.TileContext(nc) as tc:
        with (
            tc.tile_pool(name="wp", bufs=1) as wp,
            tc.tile_pool(name="sp", bufs=1) as sp,
            tc.tile_pool(name="wk", bufs=2) as wk,
            tc.tile_pool(name="stp", bufs=2) as stp,
            tc.tile_pool(name="pst", bufs=4, space="PSUM") as pst,
            tc.tile_pool(name="ps1", bufs=2, space="PSUM") as ps1p,
            tc.tile_pool(name="ps2", bufs=2, space="PSUM") as ps2p,
        ):
            # ---- persistent weights / constants ----
            w1 = [wp.tile([128, 2, DFF], f8, name=f"w1_{k}")
                  for k in range(KP)]
            w2 = [wp.tile([128, 2, DH], f8, name=f"w2_{f}")
                  for f in range(FP)]
            hwt = [wp.tile([128, DOUT], f16, name=f"hwt_{k}")
                   for k in range(KD)]
            b1s = wp.tile([128, FD], f32, name="b1s")
            hbs = wp.tile([128, OD], f32, name="hbs")
            ident = wp.tile([128, 128], f16, name="ident")

            # ---- persistent state ----
            h = [sp.tile([128, DH], f32, name=f"h_{r}") for r in range(RD)]
            xeg = [sp.tile([128, DH], f16, name=f"xeg_{r}") for r in range(RD)]
            hnT8 = [sp.tile([128, 2, R], f8, name=f"hnT8_{k}")
                    for k in range(KP)]
            hnT16 = [sp.tile([128, R], f16, name=f"hnT16_{k}")
                     for k in range(KD)]

            masks.make_identity(nc, ident[:])

            # ---- embed (transient pool, released before the step loop) ----
            with tc.tile_pool(name="ep", bufs=1) as ep:
                xts = [ep.tile([128, R], f16, name=f"xts_{i}")
                       for i in range(ID)]
                ewt = [ep.tile([128, DH], f16, name=f"ewt_{i}")
                       for i in range(ID)]
                bc1 = ep.tile([128, DH], f32, name="bc1")
                bc2 = ep.tile([128, DH], f32, name="bc2")
                for i in range(ID):
                    nc.sync.dma_start(
                        xts[i][:], xT_d.ap()[i * 128:(i + 1) * 128, :])
                    nc.sync.dma_start(
                        ewt[i][:], ewT_d.ap()[i * 128:(i + 1) * 128, :])
                nc.sync.dma_start(bc1[0:1, :], vb1_d.ap())
                nc.sync.dma_start(bc2[0:1, :], vb2_d.ap())
                nc.gpsimd.partition_broadcast(bc1[:], bc1[0:1, :])
                nc.gpsimd.partition_broadcast(bc2[:], bc2[0:1, :])

                # weight loads AFTER embed inputs: embed matmuls start
                # immediately; w1 streams in behind them
                for k in range(KP):
                    nc.sync.dma_start(
                        w1[k][:],
                        w1q_d.ap()[k * 128:(k + 1) * 128, :].rearrange(
                            "p (two f) -> p two f", two=2))
                nc.sync.dma_start(
                    b1s[:], b1e_d.ap().rearrange("(f p) o -> p (f o)", p=128))
                nc.sync.dma_start(
                    hbs[:], hb_d.ap().rearrange("(t p) o -> p (t o)", p=128))

                # H0 = 256*(x @ ewT + embed_b) ; xeg = g*H0 + 256*g*b2 (f16)
                for r in range(RD):
                    for half in range(NH):
                        sl = slice(half * 512, (half + 1) * 512)
                        pe = ps1p.tile([128, 512], f32, tag="ps1", name="pe")
                        for i in range(ID):
                            nc.tensor.matmul(
                                pe[:], xts[i][:, r * 128:(r + 1) * 128],
                                ewt[i][:, sl],
                                start=(i == 0), stop=(i == ID - 1))
                        nc.vector.tensor_tensor(
                            h[r][:, sl], pe[:], bc1[:, sl], op=OP.add)
                        nc.vector.scalar_tensor_tensor(
                            xeg[r][:, sl], h[r][:, sl], GAMMA, bc2[:, sl],
                            op0=OP.mult, op1=OP.add)

            # ---- initial LN stats on H0 (ACT sqrt once; hides table load) ----
            mv0 = stp.tile([128, RD * 2], f32, tag="mv", name="mv_init")
            for r in range(RD):
                st6 = stp.tile([128, 12], f32, tag="st6", name=f"st6_i_{r}")
                for c in range(2):
                    nc.vector.bn_stats(
                        st6[:, c * 6:(c + 1) * 6],
                        h[r][:, c * 512:(c + 1) * 512])
                nc.vector.bn_aggr(
                    mv0[:].rearrange("p (r x) -> p r x", x=2)[:, r], st6[:])
            ve0 = stp.tile([128, RD], f32, tag="ve", name="ve_init")
            nc.vector.tensor_scalar(
                ve0[:], mv0[:].rearrange("p (r x) -> p x r", x=2)[:, 1],
                LN_EPS, None, op0=OP.add)
            rv0 = stp.tile([128, RD], f32, tag="rv", name="rv_init")
            nc.vector.reciprocal(rv0[:], ve0[:])
            rs_prev = stp.tile([128, RD], f32, tag="rs", name="rs_init")
            nc.scalar.activation(rs_prev[:], rv0[:], AF.Sqrt)

            # ---- hidT pool reuses the embed pool space ----
            with tc.tile_pool(name="hp", bufs=1) as hp:
                hidT8 = [hp.tile([128, 2, R], f8, name=f"hidT8_{f}")
                         for f in range(FP)]

                def rstd_newton(y_out, y_seed, var_ap, tag_sfx, n_iter=2):
                    """y_out[128,1] = 1/sqrt(var+eps) via Newton from seed."""
                    hv = stp.tile([128, 1], f32, tag="hv",
                                  name=f"hv_{tag_sfx}")
                    nc.vector.tensor_scalar(
                        hv[:], var_ap, -0.5, -0.5 * LN_EPS,
                        op0=OP.mult, op1=OP.add)
                    y = y_seed
                    for it in range(n_iter):
                        a = stp.tile([128, 1], f32, tag="nwa",
                                     name=f"nwa_{tag_sfx}_{it}")
                        nc.vector.tensor_tensor(a[:], y, y, op=OP.mult)
                        nc.vector.tensor_scalar(
                            a[:], a[:], hv[:], 1.5, op0=OP.mult, op1=OP.add)
                        if it == n_iter - 1:
                            nc.vector.tensor_tensor(y_out, y, a[:], op=OP.mult)
                        else:
                            yn = stp.tile([128, 1], f32, tag="nwy",
                                          name=f"nwy_{tag_sfx}_{it}")
                            nc.vector.tensor_tensor(yn[:], y, a[:], op=OP.mult)
                            y = yn[:]

                # normalize H0 -> hnT8 for step 0 (one Newton polish on seed)
                rs_fix = stp.tile([128, RD], f32, tag="rsf", name="rs_fix")
                for r in range(RD):
                    rstd_newton(rs_fix[:, r:r + 1], rs_prev[:, r:r + 1],
                                mv0[:, 2 * r + 1:2 * r + 2], f"i{r}", n_iter=1)
                rs_p = rs_fix

                def normalize(r, mean_ap, rs_col, sfx):
                    nmu = stp.tile([128, 1], f32, tag="nmu", name=f"nmu_{sfx}")
                    nc.vector.scalar_tensor_tensor(
                        nmu[:], mean_ap, -1.0, rs_col,
                        op0=OP.mult, op1=OP.mult)
                    hn16 = wk.tile([128, DH], f16, tag=f"hn16_{r}",
                                   name=f"hn16_{sfx}", bufs=1)
                    # two half-width ops so transposes of the low half can
                    # start before the high half is normalized
                    for half in range(NH):
                        sl = slice(half * 512, (half + 1) * 512)
                        nc.vector.tensor_scalar(
                            hn16[:, sl], h[r][:, sl], rs_col, nmu[:],
                            op0=OP.mult, op1=OP.add)
                    return hn16

                def transposes(r, hn16, sfx, final=False):
                    for k in range(KD):
                        tp = pst.tile([128, 128], f16, tag="tp",
                                      name=f"tp_{sfx}_{k}")
                        nc.tensor.transpose(
                            tp[:], hn16[:, k * 128:(k + 1) * 128], ident[:])
                        if final:
                            dst = hnT16[k][:, r * 128:(r + 1) * 128]
                        else:
                            dst = hnT8[k // 2][:, k % 2,
                                               r * 128:(r + 1) * 128]
                        if k % 2 == 0:
                            nc.vector.tensor_copy(dst, tp[:])
                        else:
                            nc.scalar.copy(dst, tp[:])

                for r in range(RD):
                    hn = normalize(r, mv0[:, 2 * r:2 * r + 1],
                                   rs_p[:, r:r + 1], f"s0_{r}")
                    transposes(r, hn, f"s0_{r}")

                for s in range(steps):
                    last = (s == steps - 1)
                    # hidT = tanh(W1q' @ hnT8 / 64 + b1)  (fp8 DoubleRow)
                    for f in range(FD):
                        p1 = ps1p.tile([128, 512], f32, tag="ps1",
                                       name=f"p1_{s}_{f}")
                        for kp in range(KP):
                            nc.tensor.matmul(
                                p1[:], w1[kp][:, :, f * 128:(f + 1) * 128],
                                hnT8[kp][:],
                                start=(kp == 0), stop=(kp == KP - 1),
                                perf_mode=DR)
                        nc.scalar.activation(
                            hidT8[f // 2][:, f % 2, :], p1[:], AF.Tanh,
                            bias=b1s[:, f:f + 1], scale=1.0 / WS)

                    if s == 0:
                        # w2/hwt loads deferred past step-0 mm1 so w1 gets
                        # full DMA bandwidth at startup; w2 arrives during
                        # mm1 execution, well before mm2 needs it
                        for f_ in range(FP):
                            nc.sync.dma_start(
                                w2[f_][:],
                                w2q_d.ap()[f_ * 128:(f_ + 1) * 128,
                                           :].rearrange(
                                    "p (two d) -> p two d", two=2))
                        for k_ in range(KD):
                            nc.sync.dma_start(
                                hwt[k_][:],
                                hwt_d.ap()[k_ * 128:(k_ + 1) * 128, :])

                    # per row-tile: matmul2 (+xeg seeded in PSUM), update,
                    # stats, rstd, normalize, transpose -- interleaved so PE
                    # never idles at the step boundary.
                    mv = stp.tile([128, RD * 2], f32, tag="mv",
                                  name=f"mv_{s}")
                    mvv = mv[:].rearrange("p (r x) -> p r x", x=2)
                    rs = stp.tile([128, RD], f32, tag="rs", name=f"rs_{s}")
                    hns = {}
                    for r in range(RD):
                        st6 = stp.tile([128, 12], f32, tag="st6",
                                       name=f"st6_{s}_{r}")
                        for half in range(NH):
                            sl = slice(half * 512, (half + 1) * 512)
                            p2 = ps2p.tile([128, 512], f32, tag="ps2",
                                           name=f"p2_{s}_{r}_{half}")
                            nc.tensor.matmul(
                                p2[:], ident[:], xeg[r][:, sl],
                                start=True, stop=False)
                            for fp_ in range(FP):
                                nc.tensor.matmul(
                                    p2[:],
                                    hidT8[fp_][:, :,
                                               r * 128:(r + 1) * 128],
                                    w2[fp_][:, :, sl],
                                    start=False, stop=(fp_ == FP - 1),
                                    perf_mode=DR)
                            nc.vector.scalar_tensor_tensor(
                                h[r][:, sl], h[r][:, sl], 1.0 - GAMMA, p2[:],
                                op0=OP.mult, op1=OP.add)
                            if not last:
                                # stats chunk for this half right away
                                nc.vector.bn_stats(
                                    st6[:, half * 6:(half + 1) * 6],
                                    h[r][:, sl])
                        if last:
                            # head prep inline: cast final h to fp16 so its
                            # transposes overlap the remaining matmul2 groups
                            hc16 = wk.tile([128, DH], f16, tag=f"hn16_{r}",
                                           name=f"hc16_{r}", bufs=1)
                            nc.vector.tensor_copy(hc16[:], h[r][:])
                            hns[r] = hc16
                            continue
                        nc.vector.bn_aggr(mvv[:, r], st6[:])
                        rstd_newton(rs[:, r:r + 1], rs_p[:, r:r + 1],
                                    mv[:, 2 * r + 1:2 * r + 2], f"{s}_{r}")
                        hns[r] = normalize(r, mv[:, 2 * r:2 * r + 1],
                                           rs[:, r:r + 1], f"{s}_{r}")
                    # transposes LAST: PE has cover work while the final
                    # row-tile's DVE chain drains, so it never idles
                    for r in range(RD):
                        transposes(r, hns[r], f"{s}_{r}", final=last)
                    rs_p = rs

                # ---- head: outT = head_w @ h.T + head_b ----
                # (hnT16 holds final h transposed, prepped in-loop)
                for ot in range(OD):
                    po = ps1p.tile([128, 512], f32, tag="ps1", name=f"po_{ot}")
                    for k in range(KD):
                        nc.tensor.matmul(
                            po[:], hwt[k][:, ot * 128:(ot + 1) * 128],
                            hnT16[k][:],
                            start=(k == 0), stop=(k == KD - 1))
                    osb = wk.tile([128, 512], f32, tag="osb",
                                  name=f"osb_{ot}", bufs=1)
                    nc.scalar.activation(
                        osb[:], po[:], AF.Identity, bias=hbs[:, ot:ot + 1],
                        scale=HSC / ASC)
                    nc.sync.dma_start(
                        outT_d.ap()[ot * 128:(ot + 1) * 128, :], osb[:])

    nc.compile()
    return nc


def _get_compiled(steps: int):
    key = ("prog", steps)
    if key not in _CACHE:
        from concourse.bass_interp import get_hw_module
        nc = _build_program(steps)
        nc.m = get_hw_module(nc.m)
        _CACHE[key] = nc
    return _CACHE[key]


def _spectral_sigma(W: np.ndarray) -> float:
    W = W.astype(np.float64)
    v = np.full(W.shape[1], 1.0 / np.sqrt(W.shape[1]))
    u = W @ v
    u = u / (np.linalg.norm(u) + 1e-12)
    for _ in range(15):
        u = W @ v
        u = u / (np.linalg.norm(u) + 1e-12)
        v = W.T @ u
        v = v / (np.linalg.norm(v) + 1e-12)
    return float(u @ (W @ v))


def _q8(x: np.ndarray) -> np.ndarray:
    import ml_dtypes
    return np.clip(x, -240.0, 240.0).astype(ml_dtypes.float8_e4m3)


def _pair_layout(WT: np.ndarray, npair: int) -> np.ndarray:
    """[npair*2*128, F] -> [npair*128, 2*F]: k-tile pairs side by side."""
    F = WT.shape[1]
    return np.ascontiguousarray(
        WT.reshape(npair, 2, 128, F).transpose(0, 2, 1, 3).reshape(
            npair * 128, 2 * F))


def _prep_host(inputs: dict) -> tuple[dict, list]:
    f = {k: np.asarray(v, dtype=np.float32) for k, v in inputs.items()}
    x, ew, eb = f["x"], f["embed_w"], f["embed_b"]
    W1, b1, W2, b2 = f["W1"], f["b1"], f["W2"], f["b2"]
    ln_g, ln_b = f["ln_g"], f["ln_b"]
    hw_, hb = f["head_w"], f["head_b"]

    s1 = _spectral_sigma(W1)
    s2 = _spectral_sigma(W2)
    W1n = (W1.astype(np.float64) / s1)
    W2n = (W2.astype(np.float64) / s2)
    # fold ln gain into W1, ln bias into b1
    W1eff = W1n * ln_g.astype(np.float64)[None, :]
    b1eff = (b1.astype(np.float64) + W1n @ ln_b.astype(np.float64))
    W2eff = (GAMMA * ASC) * W2n  # h-state scale folded in

    shared = {
        "ewT": np.ascontiguousarray(ew.T * ASC).astype(np.float16),
        "w1q": _pair_layout(_q8(np.ascontiguousarray(W1eff.T) * WS), KP),
        "b1e": b1eff.astype(np.float32).reshape(DFF, 1),
        "w2q": _pair_layout(_q8(np.ascontiguousarray(W2eff.T)), FP),
        "vb1": (eb * ASC).reshape(1, DH).astype(np.float32),
        "vb2": (GAMMA * ASC * b2).reshape(1, DH).astype(np.float32),
        "hwt": np.ascontiguousarray(hw_.T / HSC).astype(np.float16),
        "hb": hb.reshape(DOUT, 1).astype(np.float32),
    }
    in_maps = []
    for c in range(N_CORES):
        shard = x[c * R:(c + 1) * R, :]
        m = dict(shared)
        m["xT"] = np.ascontiguousarray(shard.T).astype(np.float16)
        in_maps.append(m)
    return shared, in_maps


def kernel(**inputs) -> np.ndarray:
    from concourse import bass_utils
    nc = _get_compiled(STEPS)
    _, in_maps = _prep_host(inputs)
    res = None
    for attempt in range(3):
        try:
            res = bass_utils.run_bass_kernel_spmd(
                nc, in_maps, core_ids=list(range(N_CORES)))
            break
        except Exception:
            # transient NRT_EXEC_UNIT_UNRECOVERABLE device wedges clear on
            # retry
            if attempt == 2:
                raise
    out = np.empty((B, DOUT), np.float32)
    for c in range(N_CORES):
        out[c * R:(c + 1) * R, :] = res.results[c]["outT"].T
    return out


if __name__ == "__main__":
    rng = np.random.default_rng(0)
    demo = {
        "x": rng.standard_normal((B, DIN)).astype(np.float32),
        "embed_w": (rng.standard_normal((DH, DIN)) * 0.02).astype(np.float32),
        "embed_b": np.zeros(DH, np.float32),
        "W1": (rng.standard_normal((DFF, DH)) * 0.02).astype(np.float32),
        "b1": np.zeros(DFF, np.float32),
        "W2": (rng.standard_normal((DH, DFF)) * 0.02).astype(np.float32),
        "b2": np.zeros(DH, np.float32),
        "ln_g": np.ones(DH, np.float32),
        "ln_b": np.zeros(DH, np.float32),
        "head_w": (rng.standard_normal((DOUT, DH)) * 0.02).astype(np.float32),
        "head_b": np.zeros(DOUT, np.float32),
    }
    out = kernel(**demo)
    print("out", out.shape, out.dtype, float(np.abs(out).max()))


# revision 10
# speedup vs baseline: 1.6450x; 1.1332x over previous
"""Bass/Tile TRN2 kernel for nn_DimensionScaledEqProp.

Data-parallel over batch: x rows sharded across 8 NeuronCores, weights
replicated. Per-core state (h) stays resident in SBUF across the 30
sequential steps.

fp8 (e4m3) DoubleRow matmuls for the two big per-step GEMMs (2x PE
throughput vs fp16). Numerics plan:
  - h state carried scaled by 2^8 (H = 256*h) so W2eff*256 quantizes to
    fp8 in a good exponent range; LN is scale-invariant so hn == LN(h).
  - W1eff scaled by 2^6 into fp8; descale folded into ACT tanh scale.
  - hn/hid quantized to fp8 on the fly (cast copies / ACT output).
  - PSUM accumulation fp32; h state fp32; LN/update path fp32.
  - head matmul fp16 with head_w/16 and ACT post-scale 2^-4/2^-8.
CPU-simulated rel err 1.4e-2 (gate 2e-2).

Self-contained: hardcodes shapes; host side does sharding, spectral-norm
sigma (tiny: 60 matvecs), weight folding/quantization, and the final
gather/transpose.
"""
import sys
import numpy as np

for _p in ("/opt/trn_rl_repo", "/root/.axon_site/_ro/trn_rl_repo"):
    if _p not in sys.path:
        sys.path.append(_p)

B, DIN, DH, DOUT = 4096, 512, 1024, 256
DFF = 4 * DH
STEPS = 30
N_CORES = 8
R = B // N_CORES  # rows per core = 512
GAMMA = 0.5 * min(1.0, float(np.sqrt(64.0 / DIN)))
LN_EPS = 1e-5

KD = DH // 128    # 8  k-tiles over DH
FD = DFF // 128   # 32 f-tiles over DFF
RD = R // 128     # 4  row-tiles per core
ID = DIN // 128   # 4  k-tiles over DIN
OD = DOUT // 128  # 2  out-tiles over DOUT
NH = DH // 512    # 2  psum halves over DH
KP = KD // 2      # 4  k-pairs over DH (DoubleRow)
FP = FD // 2      # 16 f-pairs over DFF (DoubleRow)

ASC = 256.0       # h-state scale 2^8
WS = 64.0         # W1 fp8 scale 2^6
HSC = 16.0        # head weight scale 2^4

_CACHE = {}


def _build_program(steps: int):
    import concourse.bass as bass
    import concourse.bacc as bacc
    import concourse.mybir as mybir
    from concourse import tile, masks

    f8 = mybir.dt.float8e4
    f16 = mybir.dt.float16
    f32 = mybir.dt.float32
    AF = mybir.ActivationFunctionType
    OP = mybir.AluOpType
    DR = mybir.MatmulPerfMode.DoubleRow

    nc = bacc.Bacc("TRN2", target_bir_lowering=False, debug=False,
                   enable_asserts=True, num_devices=N_CORES)

    xT_d = nc.dram_tensor("xT", [DIN, R], f16, kind="ExternalInput")
    ewT_d = nc.dram_tensor("ewT", [DIN, DH], f16, kind="ExternalInput")
    w1q_d = nc.dram_tensor("w1q", [KP * 128, 2 * DFF], f8,
                           kind="ExternalInput")
    b1e_d = nc.dram_tensor("b1e", [DFF, 1], f32, kind="ExternalInput")
    w2q_d = nc.dram_tensor("w2q", [FP * 128, 2 * DH], f8,
                           kind="ExternalInput")
    vb1_d = nc.dram_tensor("vb1", [1, DH], f32, kind="ExternalInput")
    vb2_d = nc.dram_tensor("vb2", [1, DH], f32, kind="ExternalInput")
    hwt_d = nc.dram_tensor("hwt", [DH, DOUT], f16, kind="ExternalInput")
    hb_d = nc.dram_tensor("hb", [DOUT, 1], f32, kind="ExternalInput")
    outT_d = nc.dram_tensor("outT", [DOUT, R], f32, kind="ExternalOutput")

    with tile.TileContext(nc) as tc:
        with (
            tc.tile_pool(name="wp", bufs=1) as wp,
            tc.tile_pool(name="sp", bufs=1) as sp,
            tc.tile_pool(name="wk", bufs=2) as wk,
            tc.tile_pool(name="stp", bufs=2) as stp,
            tc.tile_pool(name="pst", bufs=2, space="PSUM") as pst,
            tc.tile_pool(name="ps1", bufs=4, space="PSUM") as ps1p,
            tc.tile_pool(name="ps2", bufs=2, space="PSUM") as ps2p,
        ):
            # ---- persistent weights / constants ----
            w1 = [wp.tile([128, 2, DFF], f8, name=f"w1_{k}")
                  for k in range(KP)]
            w2 = [wp.tile([128, 2, DH], f8, name=f"w2_{f}")
                  for f in range(FP)]
            hwt = [wp.tile([128, DOUT], f16, name=f"hwt_{k}")
                   for k in range(KD)]
            b1s = wp.tile([128, FD], f32, name="b1s")
            hbs = wp.tile([128, OD], f32, name="hbs")
            ident = wp.tile([128, 128], f16, name="ident")

            # ---- persistent state ----
            h = [sp.tile([128, DH], f32, name=f"h_{r}") for r in range(RD)]
            xeg = [sp.tile([128, DH], f16, name=f"xeg_{r}") for r in range(RD)]
            hnT8 = [sp.tile([128, 2, R], f8, name=f"hnT8_{k}")
                    for k in range(KP)]
            hnT16 = [sp.tile([128, R], f16, name=f"hnT16_{k}")
                     for k in range(KD)]

            masks.make_identity(nc, ident[:])

            # ---- embed (transient pool, released before the step loop) ----
            with tc.tile_pool(name="ep", bufs=1) as ep:
                xts = [ep.tile([128, R], f16, name=f"xts_{i}")
                       for i in range(ID)]
                ewt = [ep.tile([128, DH], f16, name=f"ewt_{i}")
                       for i in range(ID)]
                bc1 = ep.tile([128, DH], f32, name="bc1")
                bc2 = ep.tile([128, DH], f32, name="bc2")
                for i in range(ID):
                    nc.sync.dma_start(
                        xts[i][:], xT_d.ap()[i * 128:(i + 1) * 128, :])
                    nc.sync.dma_start(
                        ewt[i][:], ewT_d.ap()[i * 128:(i + 1) * 128, :])
                nc.sync.dma_start(bc1[0:1, :], vb1_d.ap())
                nc.sync.dma_start(bc2[0:1, :], vb2_d.ap())
                nc.gpsimd.partition_broadcast(bc1[:], bc1[0:1, :])
                nc.gpsimd.partition_broadcast(bc2[:], bc2[0:1, :])

                # weight loads AFTER embed inputs: embed matmuls start
                # immediately; w1 streams in behind them. Chunked by
                # f-columns so step-0 mm1 starts after ~1MB, not 4MB.
                nc.sync.dma_start(
                    b1s[:], b1e_d.ap().rearrange("(f p) o -> p (f o)", p=128))
                nc.sync.dma_start(
                    hbs[:], hb_d.ap().rearrange("(t p) o -> p (t o)", p=128))
                W1CH = 4
                for c in range(W1CH):
                    cw = DFF // W1CH
                    for k in range(KP):
                        nc.sync.dma_start(
                            w1[k][:, :, c * cw:(c + 1) * cw],
                            w1q_d.ap()[k * 128:(k + 1) * 128, :].rearrange(
                                "p (two f) -> p two f",
                                two=2)[:, :, c * cw:(c + 1) * cw])

                # H0 = 256*(x @ ewT + embed_b) ; xeg = g*H0 + 256*g*b2 (f16)
                for r in range(RD):
                    for half in range(NH):
                        sl = slice(half * 512, (half + 1) * 512)
                        pe = ps1p.tile([128, 512], f32, tag="ps1", name="pe")
                        for i in range(ID):
                            nc.tensor.matmul(
                                pe[:], xts[i][:, r * 128:(r + 1) * 128],
                                ewt[i][:, sl],
                                start=(i == 0), stop=(i == ID - 1))
                        nc.vector.tensor_tensor(
                            h[r][:, sl], pe[:], bc1[:, sl], op=OP.add)
                        nc.vector.scalar_tensor_tensor(
                            xeg[r][:, sl], h[r][:, sl], GAMMA, bc2[:, sl],
                            op0=OP.mult, op1=OP.add)

            # ---- initial LN stats on H0 (ACT sqrt once; hides table load) ----
            mv0 = stp.tile([128, RD * 2], f32, tag="mv", name="mv_init")
            for r in range(RD):
                st6 = stp.tile([128, 12], f32, tag="st6", name=f"st6_i_{r}")
                for c in range(2):
                    nc.vector.bn_stats(
                        st6[:, c * 6:(c + 1) * 6],
                        h[r][:, c * 512:(c + 1) * 512])
                nc.vector.bn_aggr(
                    mv0[:].rearrange("p (r x) -> p r x", x=2)[:, r], st6[:])
            ve0 = stp.tile([128, RD], f32, tag="ve", name="ve_init")
            nc.vector.tensor_scalar(
                ve0[:], mv0[:].rearrange("p (r x) -> p x r", x=2)[:, 1],
                LN_EPS, None, op0=OP.add)
            rv0 = stp.tile([128, RD], f32, tag="rv", name="rv_init")
            nc.vector.reciprocal(rv0[:], ve0[:])
            rs_prev = stp.tile([128, RD], f32, tag="rs", name="rs_init")
            nc.scalar.activation(rs_prev[:], rv0[:], AF.Sqrt)

            # ---- hidT pool reuses the embed pool space ----
            with tc.tile_pool(name="hp", bufs=1) as hp:
                hidT8 = [hp.tile([128, 2, R], f8, name=f"hidT8_{f}")
                         for f in range(FP)]

                def rstd_newton(y_out, y_seed, var_ap, tag_sfx, n_iter=2):
                    """y_out[128,1] = 1/sqrt(var+eps) via Newton from seed."""
                    hv = stp.tile([128, 1], f32, tag="hv",
                                  name=f"hv_{tag_sfx}")
                    nc.vector.tensor_scalar(
                        hv[:], var_ap, -0.5, -0.5 * LN_EPS,
                        op0=OP.mult, op1=OP.add)
                    y = y_seed
                    for it in range(n_iter):
                        a = stp.tile([128, 1], f32, tag="nwa",
                                     name=f"nwa_{tag_sfx}_{it}")
                        nc.vector.tensor_tensor(a[:], y, y, op=OP.mult)
                        nc.vector.tensor_scalar(
                            a[:], a[:], hv[:], 1.5, op0=OP.mult, op1=OP.add)
                        if it == n_iter - 1:
                            nc.vector.tensor_tensor(y_out, y, a[:], op=OP.mult)
                        else:
                            yn = stp.tile([128, 1], f32, tag="nwy",
                                          name=f"nwy_{tag_sfx}_{it}")
                            nc.vector.tensor_tensor(yn[:], y, a[:], op=OP.mult)
                            y = yn[:]

                # normalize H0 -> hnT8 for step 0 (one Newton polish on seed)
                rs_fix = stp.tile([128, RD], f32, tag="rsf", name="rs_fix")
                for r in range(RD):
                    rstd_newton(rs_fix[:, r:r + 1], rs_prev[:, r:r + 1],
                                mv0[:, 2 * r + 1:2 * r + 2], f"i{r}", n_iter=1)
                rs_p = rs_fix

                def normalize(r, mean_ap, rs_col, sfx, on_act=True):
                    nmu = stp.tile([128, 1], f32, tag="nmu", name=f"nmu_{sfx}")
                    nc.vector.scalar_tensor_tensor(
                        nmu[:], mean_ap, -1.0, rs_col,
                        op0=OP.mult, op1=OP.mult)
                    hn16 = wk.tile([128, DH], f16, tag=f"hn16_{r}",
                                   name=f"hn16_{sfx}", bufs=1)
                    # hn = h*rstd + (-mu*rstd); half-width ops so transposes
                    # of the low half start before the high half. Rows 0-2 on
                    # ACT (idle then); the step-critical last row on DVE.
                    for half in range(NH):
                        sl = slice(half * 512, (half + 1) * 512)
                        if on_act:
                            nc.scalar.activation(
                                hn16[:, sl], h[r][:, sl], AF.Identity,
                                bias=nmu[:], scale=rs_col)
                        else:
                            nc.vector.tensor_scalar(
                                hn16[:, sl], h[r][:, sl], rs_col, nmu[:],
                                op0=OP.mult, op1=OP.add)
                    return hn16

                def transposes(r, hn16, sfx, final=False):
                    for k in range(KD):
                        tp = pst.tile([128, 128], f16, tag="tp",
                                      name=f"tp_{sfx}_{k}")
                        nc.tensor.transpose(
                            tp[:], hn16[:, k * 128:(k + 1) * 128], ident[:])
                        if final:
                            dst = hnT16[k][:, r * 128:(r + 1) * 128]
                        else:
                            dst = hnT8[k // 2][:, k % 2,
                                               r * 128:(r + 1) * 128]
                        if k % 2 == 0:
                            nc.vector.tensor_copy(dst, tp[:])
                        else:
                            nc.scalar.copy(dst, tp[:])

                for r in range(RD):
                    hn = normalize(r, mv0[:, 2 * r:2 * r + 1],
                                   rs_p[:, r:r + 1], f"s0_{r}")
                    transposes(r, hn, f"s0_{r}")

                for s in range(steps):
                    last = (s == steps - 1)
                    # hidT = tanh(W1q' @ hnT8 / 64 + b1)  (fp8 DoubleRow)
                    # f-tiles processed in pairs on two PSUM banks with
                    # matmuls interleaved: consecutive PE instructions hit
                    # alternating banks, hiding PSUM accumulate turnaround
                    hpre = {}
                    for fq in range(FD // 2):
                        fa, fb = 2 * fq, 2 * fq + 1
                        pa = ps1p.tile([128, 512], f32, tag="ps1",
                                       name=f"p1_{s}_{fa}")
                        pb = ps1p.tile([128, 512], f32, tag="ps1",
                                       name=f"p1_{s}_{fb}")
                        for kp in range(KP):
                            nc.tensor.matmul(
                                pa[:], w1[kp][:, :, fa * 128:(fa + 1) * 128],
                                hnT8[kp][:],
                                start=(kp == 0), stop=(kp == KP - 1),
                                perf_mode=DR)
                            nc.tensor.matmul(
                                pb[:], w1[kp][:, :, fb * 128:(fb + 1) * 128],
                                hnT8[kp][:],
                                start=(kp == 0), stop=(kp == KP - 1),
                                perf_mode=DR)
                        nc.scalar.activation(
                            hidT8[fq][:, 0, :], pa[:], AF.Tanh,
                            bias=b1s[:, fa:fa + 1], scale=1.0 / WS)
                        nc.scalar.activation(
                            hidT8[fq][:, 1, :], pb[:], AF.Tanh,
                            bias=b1s[:, fb:fb + 1], scale=1.0 / WS)
                        # hpre = (1-g)*h + xeg on DVE, hidden under mm1;
                        # replaces the per-group fp16 PSUM-seed matmuls
                        if fq < RD:
                            r = fq
                            hp_t = wk.tile([128, DH], f32, tag=f"hpre_{r}",
                                           name=f"hpre_{s}_{r}", bufs=1)
                            nc.vector.scalar_tensor_tensor(
                                hp_t[:], h[r][:], 1.0 - GAMMA, xeg[r][:],
                                op0=OP.mult, op1=OP.add)
                            hpre[r] = hp_t

                    if s == 0:
                        # w2/hwt loads deferred past step-0 mm1 so w1 gets
                        # full DMA bandwidth at startup; w2 arrives during
                        # mm1 execution, well before mm2 needs it. dh-half
                        # chunks: half 0 of every f-pair lands first.
                        for hf in range(NH):
                            for f_ in range(FP):
                                nc.sync.dma_start(
                                    w2[f_][:, :, hf * 512:(hf + 1) * 512],
                                    w2q_d.ap()[f_ * 128:(f_ + 1) * 128,
                                               :].rearrange(
                                        "p (two d) -> p two d",
                                        two=2)[:, :, hf * 512:(hf + 1) * 512])
                        for k_ in range(KD):
                            nc.sync.dma_start(
                                hwt[k_][:],
                                hwt_d.ap()[k_ * 128:(k_ + 1) * 128, :])

                    # per row-tile: matmul2 over both dh-halves on two PSUM
                    # banks (interleaved), then update, stats, rstd,
                    # normalize, transpose -- so PE never idles at the
                    # step boundary.
                    mv = stp.tile([128, RD * 2], f32, tag="mv",
                                  name=f"mv_{s}")
                    mvv = mv[:].rearrange("p (r x) -> p r x", x=2)
                    rs = stp.tile([128, RD], f32, tag="rs", name=f"rs_{s}")
                    hns = {}
                    for r in range(RD):
                        st6 = stp.tile([128, 12], f32, tag="st6",
                                       name=f"st6_{s}_{r}")
                        pa = ps2p.tile([128, 512], f32, tag="ps2",
                                       name=f"p2_{s}_{r}_0")
                        pb = ps2p.tile([128, 512], f32, tag="ps2",
                                       name=f"p2_{s}_{r}_1")
                        rsl = slice(r * 128, (r + 1) * 128)
                        if r < RD - 1:
                            # interleaved halves: alternating PSUM banks
                            for fp_ in range(FP):
                                nc.tensor.matmul(
                                    pa[:], hidT8[fp_][:, :, rsl],
                                    w2[fp_][:, :, 0:512],
                                    start=(fp_ == 0), stop=(fp_ == FP - 1),
                                    perf_mode=DR)
                                nc.tensor.matmul(
                                    pb[:], hidT8[fp_][:, :, rsl],
                                    w2[fp_][:, :, 512:1024],
                                    start=(fp_ == 0), stop=(fp_ == FP - 1),
                                    perf_mode=DR)
                            for half, pp in ((0, pa), (1, pb)):
                                sl = slice(half * 512, (half + 1) * 512)
                                nc.vector.tensor_tensor(
                                    h[r][:, sl], hpre[r][:, sl], pp[:],
                                    op=OP.add)
                                if not last:
                                    nc.vector.bn_stats(
                                        st6[:, half * 6:(half + 1) * 6],
                                        h[r][:, sl])
                        else:
                            # last row: sequential halves so half 0's
                            # update+stats hide under half 1's matmuls,
                            # shortening the step-boundary tail
                            for half, pp in ((0, pa), (1, pb)):
                                sl = slice(half * 512, (half + 1) * 512)
                                for fp_ in range(FP):
                                    nc.tensor.matmul(
                                        pp[:], hidT8[fp_][:, :, rsl],
                                        w2[fp_][:, :, sl],
                                        start=(fp_ == 0),
                                        stop=(fp_ == FP - 1),
                                        perf_mode=DR)
                                nc.vector.tensor_tensor(
                                    h[r][:, sl], hpre[r][:, sl], pp[:],
                                    op=OP.add)
                                if not last:
                                    nc.vector.bn_stats(
                                        st6[:, half * 6:(half + 1) * 6],
                                        h[r][:, sl])
                        if last:
                            # head prep inline: cast final h to fp16 so its
                            # transposes overlap the remaining matmul2 groups
                            hc16 = wk.tile([128, DH], f16, tag=f"hn16_{r}",
                                           name=f"hc16_{r}", bufs=1)
                            nc.vector.tensor_copy(hc16[:], h[r][:])
                            hns[r] = hc16
                            continue
                        nc.vector.bn_aggr(mvv[:, r], st6[:])
                        rstd_newton(rs[:, r:r + 1], rs_p[:, r:r + 1],
                                    mv[:, 2 * r + 1:2 * r + 2], f"{s}_{r}")
                        hns[r] = normalize(r, mv[:, 2 * r:2 * r + 1],
                                           rs[:, r:r + 1], f"{s}_{r}",
                                           on_act=(r < RD - 1))
                    # transposes LAST: PE has cover work while the final
                    # row-tile's DVE chain drains, so it never idles
                    for r in range(RD):
                        transposes(r, hns[r], f"{s}_{r}", final=last)
                    rs_p = rs

                # ---- head: outT = head_w @ h.T + head_b ----
                # (hnT16 holds final h transposed, prepped in-loop)
                for ot in range(OD):
                    po = ps1p.tile([128, 512], f32, tag="ps1", name=f"po_{ot}")
                    for k in range(KD):
                        nc.tensor.matmul(
                            po[:], hwt[k][:, ot * 128:(ot + 1) * 128],
                            hnT16[k][:],
                            start=(k == 0), stop=(k == KD - 1))
                    osb = wk.tile([128, 512], f32, tag="osb",
                                  name=f"osb_{ot}", bufs=1)
                    nc.scalar.activation(
                        osb[:], po[:], AF.Identity, bias=hbs[:, ot:ot + 1],
                        scale=HSC / ASC)
                    nc.sync.dma_start(
                        outT_d.ap()[ot * 128:(ot + 1) * 128, :], osb[:])

    nc.compile()
    return nc


def _get_compiled(steps: int):
    key = ("prog", steps)
    if key not in _CACHE:
        from concourse.bass_interp import get_hw_module
        nc = _build_program(steps)
        nc.m = get_hw_module(nc.m)
        _CACHE[key] = nc
    return _CACHE[key]


def _spectral_sigma(W: np.ndarray) -> float:
    W = W.astype(np.float64)
    v = np.full(W.shape[1], 1.0 / np.sqrt(W.shape[1]))
    u = W @ v
    u = u / (np.linalg.norm(u) + 1e-12)
    for _ in range(15):
        u = W @ v
        u = u / (np.linalg.norm(u) + 1e-12)
        v = W.T @ u
        v = v / (np.linalg.norm(v) + 1e-12)
    return float(u @ (W @ v))


def _q8(x: np.ndarray) -> np.ndarray:
    import ml_dtypes
    return np.clip(x, -240.0, 240.0).astype(ml_dtypes.float8_e4m3)


def _pair_layout(WT: np.ndarray, npair: int) -> np.ndarray:
    """[npair*2*128, F] -> [npair*128, 2*F]: k-tile pairs side by side."""
    F = WT.shape[1]
    return np.ascontiguousarray(
        WT.reshape(npair, 2, 128, F).transpose(0, 2, 1, 3).reshape(
            npair * 128, 2 * F))


def _prep_host(inputs: dict) -> tuple[dict, list]:
    f = {k: np.asarray(v, dtype=np.float32) for k, v in inputs.items()}
    x, ew, eb = f["x"], f["embed_w"], f["embed_b"]
    W1, b1, W2, b2 = f["W1"], f["b1"], f["W2"], f["b2"]
    ln_g, ln_b = f["ln_g"], f["ln_b"]
    hw_, hb = f["head_w"], f["head_b"]

    s1 = _spectral_sigma(W1)
    s2 = _spectral_sigma(W2)
    W1n = (W1.astype(np.float64) / s1)
    W2n = (W2.astype(np.float64) / s2)
    # fold ln gain into W1, ln bias into b1
    W1eff = W1n * ln_g.astype(np.float64)[None, :]
    b1eff = (b1.astype(np.float64) + W1n @ ln_b.astype(np.float64))
    W2eff = (GAMMA * ASC) * W2n  # h-state scale folded in

    shared = {
        "ewT": np.ascontiguousarray(ew.T * ASC).astype(np.float16),
        "w1q": _pair_layout(_q8(np.ascontiguousarray(W1eff.T) * WS), KP),
        "b1e": b1eff.astype(np.float32).reshape(DFF, 1),
        "w2q": _pair_layout(_q8(np.ascontiguousarray(W2eff.T)), FP),
        "vb1": (eb * ASC).reshape(1, DH).astype(np.float32),
        "vb2": (GAMMA * ASC * b2).reshape(1, DH).astype(np.float32),
        "hwt": np.ascontiguousarray(hw_.T / HSC).astype(np.float16),
        "hb": hb.reshape(DOUT, 1).astype(np.float32),
    }
    in_maps = []
    for c in range(N_CORES):
        shard = x[c * R:(c + 1) * R, :]
        m = dict(shared)
        m["xT"] = np.ascontiguousarray(shard.T).astype(np.float16)
        in_maps.append(m)
    return shared, in_maps


def kernel(**inputs) -> np.ndarray:
    from concourse import bass_utils
    nc = _get_compiled(STEPS)
    _, in_maps = _prep_host(inputs)
    res = None
    for attempt in range(3):
        try:
            res = bass_utils.run_bass_kernel_spmd(
                nc, in_maps, core_ids=list(range(N_CORES)))
            break
        except Exception:
            # transient NRT_EXEC_UNIT_UNRECOVERABLE device wedges clear on
            # retry
            if attempt == 2:
                raise
    out = np.empty((B, DOUT), np.float32)
    for c in range(N_CORES):
        out[c * R:(c + 1) * R, :] = res.results[c]["outT"].T
    return out


if __name__ == "__main__":
    rng = np.random.default_rng(0)
    demo = {
        "x": rng.standard_normal((B, DIN)).astype(np.float32),
        "embed_w": (rng.standard_normal((DH, DIN)) * 0.02).astype(np.float32),
        "embed_b": np.zeros(DH, np.float32),
        "W1": (rng.standard_normal((DFF, DH)) * 0.02).astype(np.float32),
        "b1": np.zeros(DFF, np.float32),
        "W2": (rng.standard_normal((DH, DFF)) * 0.02).astype(np.float32),
        "b2": np.zeros(DH, np.float32),
        "ln_g": np.ones(DH, np.float32),
        "ln_b": np.zeros(DH, np.float32),
        "head_w": (rng.standard_normal((DOUT, DH)) * 0.02).astype(np.float32),
        "head_b": np.zeros(DOUT, np.float32),
    }
    out = kernel(**demo)
    print("out", out.shape, out.dtype, float(np.abs(out).max()))


# revision 15
# speedup vs baseline: 1.7008x; 1.0339x over previous
"""Bass/Tile TRN2 kernel for nn_DimensionScaledEqProp.

Data-parallel over batch: x rows sharded across 8 NeuronCores, weights
replicated. Per-core state (h) stays resident in SBUF across the 30
sequential steps.

fp8 (e4m3) DoubleRow matmuls for the two big per-step GEMMs (2x PE
throughput vs fp16). Numerics plan:
  - h state carried scaled by 2^8 (H = 256*h) so W2eff*256 quantizes to
    fp8 in a good exponent range; LN is scale-invariant so hn == LN(h).
  - W1eff scaled by 2^6 into fp8; descale folded into ACT tanh scale.
  - hn/hid quantized to fp8 on the fly (cast copies / ACT output).
  - PSUM accumulation fp32; h state fp32; LN/update path fp32.
  - head matmul fp16 with head_w/16 and ACT post-scale 2^-4/2^-8.
CPU-simulated rel err 1.4e-2 (gate 2e-2).

Self-contained: hardcodes shapes; host side does sharding, spectral-norm
sigma (tiny: 60 matvecs), weight folding/quantization, and the final
gather/transpose.
"""
import sys
import numpy as np

for _p in ("/opt/trn_rl_repo", "/root/.axon_site/_ro/trn_rl_repo"):
    if _p not in sys.path:
        sys.path.append(_p)

B, DIN, DH, DOUT = 4096, 512, 1024, 256
DFF = 4 * DH
STEPS = 30
N_CORES = 8
R = B // N_CORES  # rows per core = 512
GAMMA = 0.5 * min(1.0, float(np.sqrt(64.0 / DIN)))
LN_EPS = 1e-5

KD = DH // 128    # 8  k-tiles over DH
FD = DFF // 128   # 32 f-tiles over DFF
RD = R // 128     # 4  row-tiles per core
ID = DIN // 128   # 4  k-tiles over DIN
OD = DOUT // 128  # 2  out-tiles over DOUT
NH = DH // 512    # 2  psum halves over DH
KP = KD // 2      # 4  k-pairs over DH (DoubleRow)
FP = FD // 2      # 16 f-pairs over DFF (DoubleRow)

ASC = 256.0       # h-state scale 2^8
WS = 64.0         # W1 fp8 scale 2^6
HSC = 16.0        # head weight scale 2^4

_CACHE = {}


def _build_program(steps: int):
    import concourse.bass as bass
    import concourse.bacc as bacc
    import concourse.mybir as mybir
    from concourse import tile, masks

    f8 = mybir.dt.float8e4
    f16 = mybir.dt.float16
    f32 = mybir.dt.float32
    AF = mybir.ActivationFunctionType
    OP = mybir.AluOpType
    DR = mybir.MatmulPerfMode.DoubleRow

    nc = bacc.Bacc("TRN2", target_bir_lowering=False, debug=False,
                   enable_asserts=True, num_devices=N_CORES)

    xT_d = nc.dram_tensor("xT", [DIN, R], f16, kind="ExternalInput")
    ewT_d = nc.dram_tensor("ewT", [DIN, DH], f16, kind="ExternalInput")
    w1q_d = nc.dram_tensor("w1q", [KP * 128, 2 * DFF], f8,
                           kind="ExternalInput")
    b1e_d = nc.dram_tensor("b1e", [DFF, 1], f32, kind="ExternalInput")
    w2q_d = nc.dram_tensor("w2q", [FP * 128, 2 * DH], f8,
                           kind="ExternalInput")
    vb1_d = nc.dram_tensor("vb1", [1, DH], f32, kind="ExternalInput")
    vb2_d = nc.dram_tensor("vb2", [1, DH], f32, kind="ExternalInput")
    hwt_d = nc.dram_tensor("hwt", [DH, DOUT], f16, kind="ExternalInput")
    hb_d = nc.dram_tensor("hb", [DOUT, 1], f32, kind="ExternalInput")
    outT_d = nc.dram_tensor("outT", [DOUT, R], f32, kind="ExternalOutput")

    with tile.TileContext(nc) as tc:
        with (
            tc.tile_pool(name="wp", bufs=1) as wp,
            tc.tile_pool(name="sp", bufs=1) as sp,
            tc.tile_pool(name="wk", bufs=2) as wk,
            tc.tile_pool(name="stp", bufs=2) as stp,
            tc.tile_pool(name="pst", bufs=3, space="PSUM") as pst,
            tc.tile_pool(name="ps1", bufs=3, space="PSUM") as ps1p,
            tc.tile_pool(name="ps2", bufs=2, space="PSUM") as ps2p,
        ):
            # ---- persistent weights / constants ----
            w1 = [wp.tile([128, 2, DFF], f8, name=f"w1_{k}")
                  for k in range(KP)]
            w2 = [wp.tile([128, 2, DH], f8, name=f"w2_{f}")
                  for f in range(FP)]
            hwt = [wp.tile([128, DOUT], f16, name=f"hwt_{k}")
                   for k in range(KD)]
            b1s = wp.tile([128, FD], f32, name="b1s")
            hbs = wp.tile([128, OD], f32, name="hbs")
            ident = wp.tile([128, 128], f16, name="ident")

            # ---- persistent state ----
            h = [sp.tile([128, DH], f32, name=f"h_{r}") for r in range(RD)]
            xeg = [sp.tile([128, DH], f16, name=f"xeg_{r}") for r in range(RD)]
            hnT8 = [sp.tile([128, 2, R], f8, name=f"hnT8_{k}")
                    for k in range(KP)]
            hnT16 = [sp.tile([128, R], f16, name=f"hnT16_{k}")
                     for k in range(KD)]

            masks.make_identity(nc, ident[:])

            # ---- embed (transient pool, released before the step loop) ----
            with tc.tile_pool(name="ep", bufs=1) as ep:
                xts = [ep.tile([128, R], f16, name=f"xts_{i}")
                       for i in range(ID)]
                ewt = [ep.tile([128, DH], f16, name=f"ewt_{i}")
                       for i in range(ID)]
                bc1 = ep.tile([128, DH], f32, name="bc1")
                bc2 = ep.tile([128, DH], f32, name="bc2")
                # embed inputs split across the two HW DMA queues (sync=SP,
                # scalar=ACT) so they land ~2x sooner
                for i in range(ID):
                    nc.sync.dma_start(
                        xts[i][:], xT_d.ap()[i * 128:(i + 1) * 128, :])
                    nc.scalar.dma_start(
                        ewt[i][:], ewT_d.ap()[i * 128:(i + 1) * 128, :])
                nc.sync.dma_start(bc1[0:1, :], vb1_d.ap())
                nc.sync.dma_start(bc2[0:1, :], vb2_d.ap())
                nc.gpsimd.partition_broadcast(bc1[:], bc1[0:1, :])
                nc.gpsimd.partition_broadcast(bc2[:], bc2[0:1, :])

                # weight loads AFTER embed inputs: embed matmuls start
                # immediately; w1 streams in behind them on the ACT queue
                # (idle at startup) while w2 takes the sync queue. w1
                # chunked by f-columns so step-0 mm1 starts after ~1MB.
                nc.sync.dma_start(
                    b1s[:], b1e_d.ap().rearrange("(f p) o -> p (f o)", p=128))
                nc.sync.dma_start(
                    hbs[:], hb_d.ap().rearrange("(t p) o -> p (t o)", p=128))
                W1CH = 4
                for c in range(W1CH):
                    cw = DFF // W1CH
                    for k in range(KP):
                        nc.scalar.dma_start(
                            w1[k][:, :, c * cw:(c + 1) * cw],
                            w1q_d.ap()[k * 128:(k + 1) * 128, :].rearrange(
                                "p (two f) -> p two f",
                                two=2)[:, :, c * cw:(c + 1) * cw])
                for hf in range(NH):
                    for f_ in range(FP):
                        nc.sync.dma_start(
                            w2[f_][:, :, hf * 512:(hf + 1) * 512],
                            w2q_d.ap()[f_ * 128:(f_ + 1) * 128, :].rearrange(
                                "p (two d) -> p two d",
                                two=2)[:, :, hf * 512:(hf + 1) * 512])
                for k_ in range(KD):
                    nc.scalar.dma_start(
                        hwt[k_][:],
                        hwt_d.ap()[k_ * 128:(k_ + 1) * 128, :])

                # H0 = 256*(x @ ewT + embed_b) ; xeg = g*H0 + 256*g*b2 (f16)
                for r in range(RD):
                    for half in range(NH):
                        sl = slice(half * 512, (half + 1) * 512)
                        pe = ps1p.tile([128, 512], f32, tag="ps1", name="pe")
                        for i in range(ID):
                            nc.tensor.matmul(
                                pe[:], xts[i][:, r * 128:(r + 1) * 128],
                                ewt[i][:, sl],
                                start=(i == 0), stop=(i == ID - 1))
                        nc.vector.tensor_tensor(
                            h[r][:, sl], pe[:], bc1[:, sl], op=OP.add)
                        nc.vector.scalar_tensor_tensor(
                            xeg[r][:, sl], h[r][:, sl], GAMMA, bc2[:, sl],
                            op0=OP.mult, op1=OP.add)

            # ---- initial LN stats on H0 (ACT sqrt once; hides table load) ----
            mv0 = stp.tile([128, RD * 2], f32, tag="mv", name="mv_init")
            for r in range(RD):
                st6 = stp.tile([128, 12], f32, tag="st6", name=f"st6_i_{r}")
                for c in range(2):
                    nc.vector.bn_stats(
                        st6[:, c * 6:(c + 1) * 6],
                        h[r][:, c * 512:(c + 1) * 512])
                nc.vector.bn_aggr(
                    mv0[:].rearrange("p (r x) -> p r x", x=2)[:, r], st6[:])
            ve0 = stp.tile([128, RD], f32, tag="ve", name="ve_init")
            nc.vector.tensor_scalar(
                ve0[:], mv0[:].rearrange("p (r x) -> p x r", x=2)[:, 1],
                LN_EPS, None, op0=OP.add)
            rv0 = stp.tile([128, RD], f32, tag="rv", name="rv_init")
            nc.vector.reciprocal(rv0[:], ve0[:])
            rs_prev = stp.tile([128, RD], f32, tag="rs", name="rs_init")
            nc.scalar.activation(rs_prev[:], rv0[:], AF.Sqrt)

            # ---- hidT pool reuses the embed pool space ----
            with tc.tile_pool(name="hp", bufs=1) as hp:
                hidT8 = [hp.tile([128, 2, R], f8, name=f"hidT8_{f}")
                         for f in range(FP)]

                def rstd_newton(y_out, y_seed, var_ap, tag_sfx, n_iter=2):
                    """y_out[128,1] = 1/sqrt(var+eps) via Newton from seed."""
                    hv = stp.tile([128, 1], f32, tag="hv",
                                  name=f"hv_{tag_sfx}")
                    nc.vector.tensor_scalar(
                        hv[:], var_ap, -0.5, -0.5 * LN_EPS,
                        op0=OP.mult, op1=OP.add)
                    y = y_seed
                    for it in range(n_iter):
                        a = stp.tile([128, 1], f32, tag="nwa",
                                     name=f"nwa_{tag_sfx}_{it}")
                        nc.vector.tensor_tensor(a[:], y, y, op=OP.mult)
                        nc.vector.tensor_scalar(
                            a[:], a[:], hv[:], 1.5, op0=OP.mult, op1=OP.add)
                        if it == n_iter - 1:
                            nc.vector.tensor_tensor(y_out, y, a[:], op=OP.mult)
                        else:
                            yn = stp.tile([128, 1], f32, tag="nwy",
                                          name=f"nwy_{tag_sfx}_{it}")
                            nc.vector.tensor_tensor(yn[:], y, a[:], op=OP.mult)
                            y = yn[:]

                # normalize H0 -> hnT8 for step 0 (one Newton polish on seed)
                rs_fix = stp.tile([128, RD], f32, tag="rsf", name="rs_fix")
                for r in range(RD):
                    rstd_newton(rs_fix[:, r:r + 1], rs_prev[:, r:r + 1],
                                mv0[:, 2 * r + 1:2 * r + 2], f"i{r}", n_iter=1)
                rs_p = rs_fix

                def normalize(r, mean_ap, rs_col, sfx):
                    nmu = stp.tile([128, 1], f32, tag="nmu", name=f"nmu_{sfx}")
                    nc.vector.scalar_tensor_tensor(
                        nmu[:], mean_ap, -1.0, rs_col,
                        op0=OP.mult, op1=OP.mult)
                    hn16 = wk.tile([128, DH], f16, tag=f"hn16_{r}",
                                   name=f"hn16_{sfx}", bufs=1)
                    # two half-width ops so transposes of the low half can
                    # start before the high half is normalized
                    for half in range(NH):
                        sl = slice(half * 512, (half + 1) * 512)
                        nc.vector.tensor_scalar(
                            hn16[:, sl], h[r][:, sl], rs_col, nmu[:],
                            op0=OP.mult, op1=OP.add)
                    return hn16

                def transposes(r, hn16, sfx, final=False):
                    # two 128x128 transposes share one PSUM tile; a single
                    # two-plane strided copy moves both into hnT8 (halves
                    # the cast-copy instruction count)
                    rsl = slice(r * 128, (r + 1) * 128)
                    for kp in range(KP):
                        tp = pst.tile([128, 256], f16, tag="tp",
                                      name=f"tp_{sfx}_{kp}")
                        for j in range(2):
                            k = 2 * kp + j
                            nc.tensor.transpose(
                                tp[:, j * 128:(j + 1) * 128],
                                hn16[:, k * 128:(k + 1) * 128], ident[:])
                        if final:
                            nc.vector.tensor_copy(
                                hnT16[2 * kp][:, rsl], tp[:, 0:128])
                            nc.scalar.copy(
                                hnT16[2 * kp + 1][:, rsl], tp[:, 128:256])
                        else:
                            src = tp[:].rearrange(
                                "p (two m) -> p two m", two=2)
                            if kp % 2 == 0:
                                nc.vector.tensor_copy(
                                    hnT8[kp][:, :, rsl], src)
                            else:
                                nc.scalar.copy(hnT8[kp][:, :, rsl], src)

                for r in range(RD):
                    hn = normalize(r, mv0[:, 2 * r:2 * r + 1],
                                   rs_p[:, r:r + 1], f"s0_{r}")
                    transposes(r, hn, f"s0_{r}")

                for s in range(steps):
                    last = (s == steps - 1)
                    # hidT = tanh(W1q' @ hnT8 / 64 + b1)  (fp8 DoubleRow)
                    # f-tiles processed in pairs on two PSUM banks with
                    # matmuls interleaved: consecutive PE instructions hit
                    # alternating banks, hiding PSUM accumulate turnaround
                    hpre = {}
                    for fq in range(FD // 2):
                        fa, fb = 2 * fq, 2 * fq + 1
                        pa = ps1p.tile([128, 512], f32, tag="ps1",
                                       name=f"p1_{s}_{fa}")
                        pb = ps1p.tile([128, 512], f32, tag="ps1",
                                       name=f"p1_{s}_{fb}")
                        for kp in range(KP):
                            nc.tensor.matmul(
                                pa[:], w1[kp][:, :, fa * 128:(fa + 1) * 128],
                                hnT8[kp][:],
                                start=(kp == 0), stop=(kp == KP - 1),
                                perf_mode=DR)
                            nc.tensor.matmul(
                                pb[:], w1[kp][:, :, fb * 128:(fb + 1) * 128],
                                hnT8[kp][:],
                                start=(kp == 0), stop=(kp == KP - 1),
                                perf_mode=DR)
                        nc.scalar.activation(
                            hidT8[fq][:, 0, :], pa[:], AF.Tanh,
                            bias=b1s[:, fa:fa + 1], scale=1.0 / WS)
                        nc.scalar.activation(
                            hidT8[fq][:, 1, :], pb[:], AF.Tanh,
                            bias=b1s[:, fb:fb + 1], scale=1.0 / WS)
                        # hpre = (1-g)*h + xeg on DVE, hidden under mm1;
                        # replaces the per-group fp16 PSUM-seed matmuls
                        if fq < RD:
                            r = fq
                            hp_t = wk.tile([128, DH], f32, tag=f"hpre_{r}",
                                           name=f"hpre_{s}_{r}", bufs=1)
                            nc.vector.scalar_tensor_tensor(
                                hp_t[:], h[r][:], 1.0 - GAMMA, xeg[r][:],
                                op0=OP.mult, op1=OP.add)
                            hpre[r] = hp_t

                    # per row-tile: matmul2 over both dh-halves on two PSUM
                    # banks (interleaved), then update, stats, rstd,
                    # normalize, transpose -- so PE never idles at the
                    # step boundary.
                    mv = stp.tile([128, RD * 2], f32, tag="mv",
                                  name=f"mv_{s}")
                    mvv = mv[:].rearrange("p (r x) -> p r x", x=2)
                    rs = stp.tile([128, RD], f32, tag="rs", name=f"rs_{s}")
                    hns = {}
                    for r in range(RD):
                        st6 = stp.tile([128, 12], f32, tag="st6",
                                       name=f"st6_{s}_{r}")
                        pa = ps2p.tile([128, 512], f32, tag="ps2",
                                       name=f"p2_{s}_{r}_0")
                        pb = ps2p.tile([128, 512], f32, tag="ps2",
                                       name=f"p2_{s}_{r}_1")
                        rsl = slice(r * 128, (r + 1) * 128)
                        if r < RD - 1:
                            # interleaved halves: alternating PSUM banks
                            for fp_ in range(FP):
                                nc.tensor.matmul(
                                    pa[:], hidT8[fp_][:, :, rsl],
                                    w2[fp_][:, :, 0:512],
                                    start=(fp_ == 0), stop=(fp_ == FP - 1),
                                    perf_mode=DR)
                                nc.tensor.matmul(
                                    pb[:], hidT8[fp_][:, :, rsl],
                                    w2[fp_][:, :, 512:1024],
                                    start=(fp_ == 0), stop=(fp_ == FP - 1),
                                    perf_mode=DR)
                            for half, pp in ((0, pa), (1, pb)):
                                sl = slice(half * 512, (half + 1) * 512)
                                nc.vector.tensor_tensor(
                                    h[r][:, sl], hpre[r][:, sl], pp[:],
                                    op=OP.add)
                                if not last:
                                    nc.vector.bn_stats(
                                        st6[:, half * 6:(half + 1) * 6],
                                        h[r][:, sl])
                        else:
                            # last row: sequential halves so half 0's
                            # update+stats hide under half 1's matmuls,
                            # shortening the step-boundary tail
                            for half, pp in ((0, pa), (1, pb)):
                                sl = slice(half * 512, (half + 1) * 512)
                                for fp_ in range(FP):
                                    nc.tensor.matmul(
                                        pp[:], hidT8[fp_][:, :, rsl],
                                        w2[fp_][:, :, sl],
                                        start=(fp_ == 0),
                                        stop=(fp_ == FP - 1),
                                        perf_mode=DR)
                                nc.vector.tensor_tensor(
                                    h[r][:, sl], hpre[r][:, sl], pp[:],
                                    op=OP.add)
                                if not last:
                                    nc.vector.bn_stats(
                                        st6[:, half * 6:(half + 1) * 6],
                                        h[r][:, sl])
                        if last:
                            # head prep inline: cast final h to fp16 so its
                            # transposes overlap the remaining matmul2 groups
                            hc16 = wk.tile([128, DH], f16, tag=f"hn16_{r}",
                                           name=f"hc16_{r}", bufs=1)
                            nc.vector.tensor_copy(hc16[:], h[r][:])
                            hns[r] = hc16
                            continue
                        nc.vector.bn_aggr(mvv[:, r], st6[:])
                        rstd_newton(rs[:, r:r + 1], rs_p[:, r:r + 1],
                                    mv[:, 2 * r + 1:2 * r + 2], f"{s}_{r}")
                        hns[r] = normalize(r, mv[:, 2 * r:2 * r + 1],
                                           rs[:, r:r + 1], f"{s}_{r}")
                    # transposes LAST: PE has cover work while the final
                    # row-tile's DVE chain drains, so it never idles
                    for r in range(RD):
                        transposes(r, hns[r], f"{s}_{r}", final=last)
                    rs_p = rs

                # ---- head: outT = head_w @ h.T + head_b ----
                # (hnT16 holds final h transposed, prepped in-loop)
                for ot in range(OD):
                    po = ps1p.tile([128, 512], f32, tag="ps1", name=f"po_{ot}")
                    for k in range(KD):
                        nc.tensor.matmul(
                            po[:], hwt[k][:, ot * 128:(ot + 1) * 128],
                            hnT16[k][:],
                            start=(k == 0), stop=(k == KD - 1))
                    osb = wk.tile([128, 512], f32, tag="osb",
                                  name=f"osb_{ot}", bufs=1)
                    nc.scalar.activation(
                        osb[:], po[:], AF.Identity, bias=hbs[:, ot:ot + 1],
                        scale=HSC / ASC)
                    nc.sync.dma_start(
                        outT_d.ap()[ot * 128:(ot + 1) * 128, :], osb[:])

    nc.compile()
    return nc


def _get_compiled(steps: int):
    key = ("prog", steps)
    if key not in _CACHE:
        from concourse.bass_interp import get_hw_module
        nc = _build_program(steps)
        nc.m = get_hw_module(nc.m)
        _CACHE[key] = nc
    return _CACHE[key]


def _spectral_sigma(W: np.ndarray) -> float:
    W = W.astype(np.float64)
    v = np.full(W.shape[1], 1.0 / np.sqrt(W.shape[1]))
    u = W @ v
    u = u / (np.linalg.norm(u) + 1e-12)
    for _ in range(15):
        u = W @ v
        u = u / (np.linalg.norm(u) + 1e-12)
        v = W.T @ u
        v = v / (np.linalg.norm(v) + 1e-12)
    return float(u @ (W @ v))


def _q8(x: np.ndarray) -> np.ndarray:
    import ml_dtypes
    return np.clip(x, -240.0, 240.0).astype(ml_dtypes.float8_e4m3)


def _pair_layout(WT: np.ndarray, npair: int) -> np.ndarray:
    """[npair*2*128, F] -> [npair*128, 2*F]: k-tile pairs side by side."""
    F = WT.shape[1]
    return np.ascontiguousarray(
        WT.reshape(npair, 2, 128, F).transpose(0, 2, 1, 3).reshape(
            npair * 128, 2 * F))


def _prep_host(inputs: dict) -> tuple[dict, list]:
    f = {k: np.asarray(v, dtype=np.float32) for k, v in inputs.items()}
    x, ew, eb = f["x"], f["embed_w"], f["embed_b"]
    W1, b1, W2, b2 = f["W1"], f["b1"], f["W2"], f["b2"]
    ln_g, ln_b = f["ln_g"], f["ln_b"]
    hw_, hb = f["head_w"], f["head_b"]

    s1 = _spectral_sigma(W1)
    s2 = _spectral_sigma(W2)
    W1n = (W1.astype(np.float64) / s1)
    W2n = (W2.astype(np.float64) / s2)
    # fold ln gain into W1, ln bias into b1
    W1eff = W1n * ln_g.astype(np.float64)[None, :]
    b1eff = (b1.astype(np.float64) + W1n @ ln_b.astype(np.float64))
    W2eff = (GAMMA * ASC) * W2n  # h-state scale folded in

    shared = {
        "ewT": np.ascontiguousarray(ew.T * ASC).astype(np.float16),
        "w1q": _pair_layout(_q8(np.ascontiguousarray(W1eff.T) * WS), KP),
        "b1e": b1eff.astype(np.float32).reshape(DFF, 1),
        "w2q": _pair_layout(_q8(np.ascontiguousarray(W2eff.T)), FP),
        "vb1": (eb * ASC).reshape(1, DH).astype(np.float32),
        "vb2": (GAMMA * ASC * b2).reshape(1, DH).astype(np.float32),
        "hwt": np.ascontiguousarray(hw_.T / HSC).astype(np.float16),
        "hb": hb.reshape(DOUT, 1).astype(np.float32),
    }
    in_maps = []
    for c in range(N_CORES):
        shard = x[c * R:(c + 1) * R, :]
        m = dict(shared)
        m["xT"] = np.ascontiguousarray(shard.T).astype(np.float16)
        in_maps.append(m)
    return shared, in_maps


def kernel(**inputs) -> np.ndarray:
    from concourse import bass_utils
    nc = _get_compiled(STEPS)
    _, in_maps = _prep_host(inputs)
    res = None
    for attempt in range(3):
        try:
            res = bass_utils.run_bass_kernel_spmd(
                nc, in_maps, core_ids=list(range(N_CORES)))
            break
        except Exception:
            # transient NRT_EXEC_UNIT_UNRECOVERABLE device wedges clear on
            # retry
            if attempt == 2:
                raise
    out = np.empty((B, DOUT), np.float32)
    for c in range(N_CORES):
        out[c * R:(c + 1) * R, :] = res.results[c]["outT"].T
    return out


if __name__ == "__main__":
    rng = np.random.default_rng(0)
    demo = {
        "x": rng.standard_normal((B, DIN)).astype(np.float32),
        "embed_w": (rng.standard_normal((DH, DIN)) * 0.02).astype(np.float32),
        "embed_b": np.zeros(DH, np.float32),
        "W1": (rng.standard_normal((DFF, DH)) * 0.02).astype(np.float32),
        "b1": np.zeros(DFF, np.float32),
        "W2": (rng.standard_normal((DH, DFF)) * 0.02).astype(np.float32),
        "b2": np.zeros(DH, np.float32),
        "ln_g": np.ones(DH, np.float32),
        "ln_b": np.zeros(DH, np.float32),
        "head_w": (rng.standard_normal((DOUT, DH)) * 0.02).astype(np.float32),
        "head_b": np.zeros(DOUT, np.float32),
    }
    out = kernel(**demo)
    print("out", out.shape, out.dtype, float(np.abs(out).max()))


# revision 23
# speedup vs baseline: 1.7095x; 1.0051x over previous
"""Bass/Tile TRN2 kernel for nn_DimensionScaledEqProp.

Data-parallel over batch: x rows sharded across 8 NeuronCores, weights
replicated. Per-core state (h) stays resident in SBUF across the 30
sequential steps.

fp8 (e4m3) DoubleRow matmuls for the two big per-step GEMMs (2x PE
throughput vs fp16). Numerics plan:
  - h state carried scaled by 2^8 (H = 256*h) so W2eff*256 quantizes to
    fp8 in a good exponent range; LN is scale-invariant so hn == LN(h).
  - W1eff scaled by 2^6 into fp8; descale folded into ACT tanh scale.
  - hn/hid quantized to fp8 on the fly (cast copies / ACT output).
  - PSUM accumulation fp32; h state fp32; LN/update path fp32.
  - head matmul fp16 with head_w/16 and ACT post-scale 2^-4/2^-8.
CPU-simulated rel err 1.4e-2 (gate 2e-2).

Self-contained: hardcodes shapes; host side does sharding, spectral-norm
sigma (tiny: 60 matvecs), weight folding/quantization, and the final
gather/transpose.
"""
import sys
import numpy as np

for _p in ("/opt/trn_rl_repo", "/root/.axon_site/_ro/trn_rl_repo"):
    if _p not in sys.path:
        sys.path.append(_p)

B, DIN, DH, DOUT = 4096, 512, 1024, 256
DFF = 4 * DH
STEPS = 30
N_CORES = 8
R = B // N_CORES  # rows per core = 512
GAMMA = 0.5 * min(1.0, float(np.sqrt(64.0 / DIN)))
LN_EPS = 1e-5

KD = DH // 128    # 8  k-tiles over DH
FD = DFF // 128   # 32 f-tiles over DFF
RD = R // 128     # 4  row-tiles per core
ID = DIN // 128   # 4  k-tiles over DIN
OD = DOUT // 128  # 2  out-tiles over DOUT
NH = DH // 512    # 2  psum halves over DH
KP = KD // 2      # 4  k-pairs over DH (DoubleRow)
FP = FD // 2      # 16 f-pairs over DFF (DoubleRow)

ASC = 256.0       # h-state scale 2^8
WS = 64.0         # W1 fp8 scale 2^6
HSC = 16.0        # head weight scale 2^4

_CACHE = {}


def _build_program(steps: int):
    import concourse.bass as bass
    import concourse.bacc as bacc
    import concourse.mybir as mybir
    from concourse import tile, masks

    f8 = mybir.dt.float8e4
    f16 = mybir.dt.float16
    f32 = mybir.dt.float32
    AF = mybir.ActivationFunctionType
    OP = mybir.AluOpType
    DR = mybir.MatmulPerfMode.DoubleRow

    nc = bacc.Bacc("TRN2", target_bir_lowering=False, debug=False,
                   enable_asserts=True, num_devices=N_CORES)

    xT_d = nc.dram_tensor("xT", [DIN, R], f16, kind="ExternalInput")
    ewT_d = nc.dram_tensor("ewT", [DIN, DH], f16, kind="ExternalInput")
    w1q_d = nc.dram_tensor("w1q", [KP * 128, 2 * DFF], f8,
                           kind="ExternalInput")
    b1e_d = nc.dram_tensor("b1e", [DFF, 1], f32, kind="ExternalInput")
    w2q_d = nc.dram_tensor("w2q", [FP * 128, 2 * DH], f8,
                           kind="ExternalInput")
    vb1_d = nc.dram_tensor("vb1", [1, DH], f32, kind="ExternalInput")
    vb2_d = nc.dram_tensor("vb2", [1, DH], f32, kind="ExternalInput")
    hwt_d = nc.dram_tensor("hwt", [DH, DOUT], f16, kind="ExternalInput")
    hb_d = nc.dram_tensor("hb", [DOUT, 1], f32, kind="ExternalInput")
    outT_d = nc.dram_tensor("outT", [DOUT, R], f32, kind="ExternalOutput")

    with tile.TileContext(nc) as tc:
        with (
            tc.tile_pool(name="wp", bufs=1) as wp,
            tc.tile_pool(name="sp", bufs=1) as sp,
            tc.tile_pool(name="wk", bufs=2) as wk,
            tc.tile_pool(name="stp", bufs=2) as stp,
            tc.tile_pool(name="pst", bufs=2, space="PSUM") as pst,
            tc.tile_pool(name="ps1", bufs=3, space="PSUM") as ps1p,
            tc.tile_pool(name="ps2", bufs=3, space="PSUM") as ps2p,
        ):
            # ---- persistent weights / constants ----
            # w1 split into per-chunk tiles: Tile dependency tracking is
            # per-tile, so separate tiles let step-0 mm1 start as soon as
            # the first f-chunk of every k-pair lands (not the full 4MB)
            W1CH = 8
            W1CW = DFF // W1CH  # 512 f-cols per chunk
            w1 = [[wp.tile([128, 2, W1CW], f8, name=f"w1_{k}_{c}")
                   for c in range(W1CH)] for k in range(KP)]
            w2 = [wp.tile([128, 2, DH], f8, name=f"w2_{f}")
                  for f in range(FP)]
            hwt = [wp.tile([128, DOUT], f16, name=f"hwt_{k}")
                   for k in range(KD)]
            b1s = wp.tile([128, FD], f32, name="b1s")
            hbs = wp.tile([128, OD], f32, name="hbs")
            ident = wp.tile([128, 128], f16, name="ident")

            # ---- persistent state ----
            h = [sp.tile([128, DH], f32, name=f"h_{r}") for r in range(RD)]
            xeg = [sp.tile([128, DH], f16, name=f"xeg_{r}") for r in range(RD)]
            hnT8 = [sp.tile([128, 2, R], f8, name=f"hnT8_{k}")
                    for k in range(KP)]
            hnT16 = [sp.tile([128, R], f16, name=f"hnT16_{k}")
                     for k in range(KD)]

            masks.make_identity(nc, ident[:])

            # ---- embed (transient pool, released before the step loop) ----
            with tc.tile_pool(name="ep", bufs=1) as ep:
                xts = [ep.tile([128, R], f16, name=f"xts_{i}")
                       for i in range(ID)]
                ewt = [ep.tile([128, DH], f16, name=f"ewt_{i}")
                       for i in range(ID)]
                bc1 = ep.tile([128, DH], f32, name="bc1")
                bc2 = ep.tile([128, DH], f32, name="bc2")
                # embed inputs split across the two HW DMA queues (sync=SP,
                # scalar=ACT) so they land ~2x sooner
                for i in range(ID):
                    nc.sync.dma_start(
                        xts[i][:], xT_d.ap()[i * 128:(i + 1) * 128, :])
                    nc.scalar.dma_start(
                        ewt[i][:], ewT_d.ap()[i * 128:(i + 1) * 128, :])
                nc.sync.dma_start(bc1[0:1, :], vb1_d.ap())
                nc.sync.dma_start(bc2[0:1, :], vb2_d.ap())
                nc.gpsimd.partition_broadcast(bc1[:], bc1[0:1, :])
                nc.gpsimd.partition_broadcast(bc2[:], bc2[0:1, :])

                # weight loads AFTER embed inputs: embed matmuls start
                # immediately; w1 streams in behind them on the ACT queue
                # (idle at startup) while w2 takes the sync queue. w1
                # chunked by f-columns so step-0 mm1 starts after ~1MB.
                nc.sync.dma_start(
                    b1s[:], b1e_d.ap().rearrange("(f p) o -> p (f o)", p=128))
                nc.sync.dma_start(
                    hbs[:], hb_d.ap().rearrange("(t p) o -> p (t o)", p=128))
                for c in range(W1CH):
                    for k in range(KP):
                        nc.scalar.dma_start(
                            w1[k][c][:],
                            w1q_d.ap()[k * 128:(k + 1) * 128, :].rearrange(
                                "p (two f) -> p two f",
                                two=2)[:, :, c * W1CW:(c + 1) * W1CW])
                for hf in range(NH):
                    for f_ in range(FP):
                        nc.sync.dma_start(
                            w2[f_][:, :, hf * 512:(hf + 1) * 512],
                            w2q_d.ap()[f_ * 128:(f_ + 1) * 128, :].rearrange(
                                "p (two d) -> p two d",
                                two=2)[:, :, hf * 512:(hf + 1) * 512])
                for k_ in range(KD):
                    nc.scalar.dma_start(
                        hwt[k_][:],
                        hwt_d.ap()[k_ * 128:(k_ + 1) * 128, :])

                # H0 = 256*(x @ ewT + embed_b) ; xeg = g*H0 + 256*g*b2 (f16)
                for r in range(RD):
                    for half in range(NH):
                        sl = slice(half * 512, (half + 1) * 512)
                        pe = ps1p.tile([128, 512], f32, tag="ps1", name="pe")
                        for i in range(ID):
                            nc.tensor.matmul(
                                pe[:], xts[i][:, r * 128:(r + 1) * 128],
                                ewt[i][:, sl],
                                start=(i == 0), stop=(i == ID - 1))
                        nc.vector.tensor_tensor(
                            h[r][:, sl], pe[:], bc1[:, sl], op=OP.add)
                        nc.vector.scalar_tensor_tensor(
                            xeg[r][:, sl], h[r][:, sl], GAMMA, bc2[:, sl],
                            op0=OP.mult, op1=OP.add)

            # ---- initial LN stats on H0 (ACT sqrt once; hides table load) ----
            mv0 = stp.tile([128, RD * 2], f32, tag="mv", name="mv_init")
            for r in range(RD):
                st6 = stp.tile([128, 12], f32, tag="st6", name=f"st6_i_{r}")
                for c in range(2):
                    nc.vector.bn_stats(
                        st6[:, c * 6:(c + 1) * 6],
                        h[r][:, c * 512:(c + 1) * 512])
                nc.vector.bn_aggr(
                    mv0[:].rearrange("p (r x) -> p r x", x=2)[:, r], st6[:])
            ve0 = stp.tile([128, RD], f32, tag="ve", name="ve_init")
            nc.vector.tensor_scalar(
                ve0[:], mv0[:].rearrange("p (r x) -> p x r", x=2)[:, 1],
                LN_EPS, None, op0=OP.add)
            rv0 = stp.tile([128, RD], f32, tag="rv", name="rv_init")
            nc.vector.reciprocal(rv0[:], ve0[:])
            rs_prev = stp.tile([128, RD], f32, tag="rs", name="rs_init")
            nc.scalar.activation(rs_prev[:], rv0[:], AF.Sqrt)

            # ---- hidT pool reuses the embed pool space ----
            with tc.tile_pool(name="hp", bufs=1) as hp:
                hidT8 = [hp.tile([128, 2, R], f8, name=f"hidT8_{f}")
                         for f in range(FP)]

                def rstd_newton(y_out, y_seed, var_ap, tag_sfx, n_iter=2):
                    """y_out[128,1] = 1/sqrt(var+eps) via Newton from seed."""
                    hv = stp.tile([128, 1], f32, tag="hv",
                                  name=f"hv_{tag_sfx}")
                    nc.vector.tensor_scalar(
                        hv[:], var_ap, -0.5, -0.5 * LN_EPS,
                        op0=OP.mult, op1=OP.add)
                    y = y_seed
                    for it in range(n_iter):
                        a = stp.tile([128, 1], f32, tag="nwa",
                                     name=f"nwa_{tag_sfx}_{it}")
                        nc.vector.tensor_tensor(a[:], y, y, op=OP.mult)
                        nc.vector.tensor_scalar(
                            a[:], a[:], hv[:], 1.5, op0=OP.mult, op1=OP.add)
                        if it == n_iter - 1:
                            nc.vector.tensor_tensor(y_out, y, a[:], op=OP.mult)
                        else:
                            yn = stp.tile([128, 1], f32, tag="nwy",
                                          name=f"nwy_{tag_sfx}_{it}")
                            nc.vector.tensor_tensor(yn[:], y, a[:], op=OP.mult)
                            y = yn[:]

                # normalize H0 -> hnT8 for step 0 (one Newton polish on seed)
                rs_fix = stp.tile([128, RD], f32, tag="rsf", name="rs_fix")
                for r in range(RD):
                    rstd_newton(rs_fix[:, r:r + 1], rs_prev[:, r:r + 1],
                                mv0[:, 2 * r + 1:2 * r + 2], f"i{r}", n_iter=1)
                rs_p = rs_fix

                def normalize(r, mean_ap, rs_col, sfx):
                    nmu = stp.tile([128, 1], f32, tag="nmu", name=f"nmu_{sfx}")
                    nc.vector.scalar_tensor_tensor(
                        nmu[:], mean_ap, -1.0, rs_col,
                        op0=OP.mult, op1=OP.mult)
                    hn16 = wk.tile([128, DH], f16, tag=f"hn16_{r}",
                                   name=f"hn16_{sfx}", bufs=1)
                    # two half-width ops so transposes of the low half can
                    # start before the high half is normalized
                    for half in range(NH):
                        sl = slice(half * 512, (half + 1) * 512)
                        nc.vector.tensor_scalar(
                            hn16[:, sl], h[r][:, sl], rs_col, nmu[:],
                            op0=OP.mult, op1=OP.add)
                    return hn16

                def transposes(r, hn16, sfx, final=False):
                    # two 128x128 transposes share one PSUM tile; a single
                    # two-plane strided copy moves both into hnT8 (halves
                    # the cast-copy instruction count)
                    rsl = slice(r * 128, (r + 1) * 128)
                    for kp in range(KP):
                        tp = pst.tile([128, 256], f16, tag="tp",
                                      name=f"tp_{sfx}_{kp}")
                        for j in range(2):
                            k = 2 * kp + j
                            nc.tensor.transpose(
                                tp[:, j * 128:(j + 1) * 128],
                                hn16[:, k * 128:(k + 1) * 128], ident[:])
                        if final:
                            nc.vector.tensor_copy(
                                hnT16[2 * kp][:, rsl], tp[:, 0:128])
                            nc.scalar.copy(
                                hnT16[2 * kp + 1][:, rsl], tp[:, 128:256])
                        else:
                            src = tp[:].rearrange(
                                "p (two m) -> p two m", two=2)
                            if kp % 2 == 0:
                                nc.vector.tensor_copy(
                                    hnT8[kp][:, :, rsl], src)
                            else:
                                nc.scalar.copy(hnT8[kp][:, :, rsl], src)

                for r in range(RD):
                    hn = normalize(r, mv0[:, 2 * r:2 * r + 1],
                                   rs_p[:, r:r + 1], f"s0_{r}")
                    transposes(r, hn, f"s0_{r}")

                for s in range(steps):
                    last = (s == steps - 1)
                    # hidT = tanh(W1q' @ hnT8 / 64 + b1)  (fp8 DoubleRow)
                    # f-tiles processed in pairs on two PSUM banks with
                    # matmuls interleaved: consecutive PE instructions hit
                    # alternating banks, hiding PSUM accumulate turnaround
                    hpre = {}

                    def w1sl(kp, ft):
                        return w1[kp][ft // 4][
                            :, :, (ft % 4) * 128:(ft % 4 + 1) * 128]

                    def mm1_pair(fq, pa, pb, lo, hi, start, stop):
                        fa, fb = 2 * fq, 2 * fq + 1
                        for kp in range(KP):
                            nc.tensor.matmul(
                                pa[:, lo:hi], w1sl(kp, fa),
                                hnT8[kp][:, :, lo:hi],
                                start=start and kp == 0,
                                stop=stop and kp == KP - 1, perf_mode=DR)
                            nc.tensor.matmul(
                                pb[:, lo:hi], w1sl(kp, fb),
                                hnT8[kp][:, :, lo:hi],
                                start=start and kp == 0,
                                stop=stop and kp == KP - 1, perf_mode=DR)

                    def mm1_act(fq, pa, pb):
                        fa, fb = 2 * fq, 2 * fq + 1
                        nc.scalar.activation(
                            hidT8[fq][:, 0, :], pa[:], AF.Tanh,
                            bias=b1s[:, fa:fa + 1], scale=1.0 / WS)
                        nc.scalar.activation(
                            hidT8[fq][:, 1, :], pb[:], AF.Tanh,
                            bias=b1s[:, fb:fb + 1], scale=1.0 / WS)

                    if s > 0:
                        # first two pairs split by row range: the rows-0..2
                        # columns depend only on already-transposed rows, so
                        # PE has cover work while the last row's stats/
                        # normalize chain drains at the step boundary; the
                        # row-3 columns follow once its transposes land
                        sp_ps = [(ps1p.tile([128, 512], f32, tag="ps1",
                                            name=f"p1_{s}_{q}_a"),
                                  ps1p.tile([128, 512], f32, tag="ps1",
                                            name=f"p1_{s}_{q}_b"))
                                 for q in range(2)]
                        for q in range(2):
                            mm1_pair(q, *sp_ps[q], 0, 384,
                                     start=True, stop=True)
                        for q in range(2):
                            mm1_pair(q, *sp_ps[q], 384, 512,
                                     start=True, stop=True)
                        for q in range(2):
                            mm1_act(q, *sp_ps[q])
                        fq_rest = range(2, FD // 2)
                    else:
                        fq_rest = range(FD // 2)
                    for j, fq in enumerate(fq_rest):
                        pa = ps1p.tile([128, 512], f32, tag="ps1",
                                       name=f"p1_{s}_{fq}_a")
                        pb = ps1p.tile([128, 512], f32, tag="ps1",
                                       name=f"p1_{s}_{fq}_b")
                        mm1_pair(fq, pa, pb, 0, 512, start=True, stop=True)
                        mm1_act(fq, pa, pb)
                        # hpre = (1-g)*h + xeg on DVE, hidden under mm1;
                        # replaces the per-group fp16 PSUM-seed matmuls
                        if j < RD:
                            hp_t = wk.tile([128, DH], f32, tag=f"hpre_{j}",
                                           name=f"hpre_{s}_{j}", bufs=1)
                            nc.vector.scalar_tensor_tensor(
                                hp_t[:], h[j][:], 1.0 - GAMMA, xeg[j][:],
                                op0=OP.mult, op1=OP.add)
                            hpre[j] = hp_t

                    # per row-tile: matmul2 over both dh-halves on two PSUM
                    # banks (interleaved), then update, stats, rstd,
                    # normalize, transpose -- so PE never idles at the
                    # step boundary.
                    mv = stp.tile([128, RD * 2], f32, tag="mv",
                                  name=f"mv_{s}")
                    mvv = mv[:].rearrange("p (r x) -> p r x", x=2)
                    rs = stp.tile([128, RD], f32, tag="rs", name=f"rs_{s}")
                    hns = {}
                    for r in range(RD):
                        st6 = stp.tile([128, 12], f32, tag="st6",
                                       name=f"st6_{s}_{r}")
                        pa = ps2p.tile([128, 512], f32, tag="ps2",
                                       name=f"p2_{s}_{r}_0")
                        pb = ps2p.tile([128, 512], f32, tag="ps2",
                                       name=f"p2_{s}_{r}_1")
                        rsl = slice(r * 128, (r + 1) * 128)
                        if r < RD - 1:
                            # interleaved halves: alternating PSUM banks
                            for fp_ in range(FP):
                                nc.tensor.matmul(
                                    pa[:], hidT8[fp_][:, :, rsl],
                                    w2[fp_][:, :, 0:512],
                                    start=(fp_ == 0), stop=(fp_ == FP - 1),
                                    perf_mode=DR)
                                nc.tensor.matmul(
                                    pb[:], hidT8[fp_][:, :, rsl],
                                    w2[fp_][:, :, 512:1024],
                                    start=(fp_ == 0), stop=(fp_ == FP - 1),
                                    perf_mode=DR)
                            for half, pp in ((0, pa), (1, pb)):
                                sl = slice(half * 512, (half + 1) * 512)
                                nc.vector.tensor_tensor(
                                    h[r][:, sl], hpre[r][:, sl], pp[:],
                                    op=OP.add)
                                if not last:
                                    nc.vector.bn_stats(
                                        st6[:, half * 6:(half + 1) * 6],
                                        h[r][:, sl])
                        else:
                            # last row: sequential halves so half 0's
                            # update+stats hide under half 1's matmuls,
                            # shortening the step-boundary tail
                            for half, pp in ((0, pa), (1, pb)):
                                sl = slice(half * 512, (half + 1) * 512)
                                for fp_ in range(FP):
                                    nc.tensor.matmul(
                                        pp[:], hidT8[fp_][:, :, rsl],
                                        w2[fp_][:, :, sl],
                                        start=(fp_ == 0),
                                        stop=(fp_ == FP - 1),
                                        perf_mode=DR)
                                nc.vector.tensor_tensor(
                                    h[r][:, sl], hpre[r][:, sl], pp[:],
                                    op=OP.add)
                                if not last:
                                    nc.vector.bn_stats(
                                        st6[:, half * 6:(half + 1) * 6],
                                        h[r][:, sl])
                        if last:
                            # head prep inline: cast final h to fp16 so its
                            # transposes overlap the remaining matmul2 groups
                            hc16 = wk.tile([128, DH], f16, tag=f"hn16_{r}",
                                           name=f"hc16_{r}", bufs=1)
                            nc.vector.tensor_copy(hc16[:], h[r][:])
                            hns[r] = hc16
                            continue
                        nc.vector.bn_aggr(mvv[:, r], st6[:])
                        rstd_newton(rs[:, r:r + 1], rs_p[:, r:r + 1],
                                    mv[:, 2 * r + 1:2 * r + 2], f"{s}_{r}")
                        hns[r] = normalize(r, mv[:, 2 * r:2 * r + 1],
                                           rs[:, r:r + 1], f"{s}_{r}")
                    # transposes LAST: PE has cover work while the final
                    # row-tile's DVE chain drains, so it never idles
                    for r in range(RD):
                        transposes(r, hns[r], f"{s}_{r}", final=last)
                    rs_p = rs

                # ---- head: outT = head_w @ h.T + head_b ----
                # (hnT16 holds final h transposed, prepped in-loop)
                for ot in range(OD):
                    po = ps1p.tile([128, 512], f32, tag="ps1", name=f"po_{ot}")
                    for k in range(KD):
                        nc.tensor.matmul(
                            po[:], hwt[k][:, ot * 128:(ot + 1) * 128],
                            hnT16[k][:],
                            start=(k == 0), stop=(k == KD - 1))
                    osb = wk.tile([128, 512], f32, tag="osb",
                                  name=f"osb_{ot}", bufs=1)
                    nc.scalar.activation(
                        osb[:], po[:], AF.Identity, bias=hbs[:, ot:ot + 1],
                        scale=HSC / ASC)
                    nc.sync.dma_start(
                        outT_d.ap()[ot * 128:(ot + 1) * 128, :], osb[:])

    nc.compile()
    return nc


def _get_compiled(steps: int):
    key = ("prog", steps)
    if key not in _CACHE:
        from concourse.bass_interp import get_hw_module
        nc = _build_program(steps)
        nc.m = get_hw_module(nc.m)
        _CACHE[key] = nc
    return _CACHE[key]


def _spectral_sigma(W: np.ndarray) -> float:
    W = W.astype(np.float64)
    v = np.full(W.shape[1], 1.0 / np.sqrt(W.shape[1]))
    u = W @ v
    u = u / (np.linalg.norm(u) + 1e-12)
    for _ in range(15):
        u = W @ v
        u = u / (np.linalg.norm(u) + 1e-12)
        v = W.T @ u
        v = v / (np.linalg.norm(v) + 1e-12)
    return float(u @ (W @ v))


def _q8(x: np.ndarray) -> np.ndarray:
    import ml_dtypes
    return np.clip(x, -240.0, 240.0).astype(ml_dtypes.float8_e4m3)


def _pair_layout(WT: np.ndarray, npair: int) -> np.ndarray:
    """[npair*2*128, F] -> [npair*128, 2*F]: k-tile pairs side by side."""
    F = WT.shape[1]
    return np.ascontiguousarray(
        WT.reshape(npair, 2, 128, F).transpose(0, 2, 1, 3).reshape(
            npair * 128, 2 * F))


def _prep_host(inputs: dict) -> tuple[dict, list]:
    f = {k: np.asarray(v, dtype=np.float32) for k, v in inputs.items()}
    x, ew, eb = f["x"], f["embed_w"], f["embed_b"]
    W1, b1, W2, b2 = f["W1"], f["b1"], f["W2"], f["b2"]
    ln_g, ln_b = f["ln_g"], f["ln_b"]
    hw_, hb = f["head_w"], f["head_b"]

    s1 = _spectral_sigma(W1)
    s2 = _spectral_sigma(W2)
    W1n = (W1.astype(np.float64) / s1)
    W2n = (W2.astype(np.float64) / s2)
    # fold ln gain into W1, ln bias into b1
    W1eff = W1n * ln_g.astype(np.float64)[None, :]
    b1eff = (b1.astype(np.float64) + W1n @ ln_b.astype(np.float64))
    W2eff = (GAMMA * ASC) * W2n  # h-state scale folded in

    shared = {
        "ewT": np.ascontiguousarray(ew.T * ASC).astype(np.float16),
        "w1q": _pair_layout(_q8(np.ascontiguousarray(W1eff.T) * WS), KP),
        "b1e": b1eff.astype(np.float32).reshape(DFF, 1),
        "w2q": _pair_layout(_q8(np.ascontiguousarray(W2eff.T)), FP),
        "vb1": (eb * ASC).reshape(1, DH).astype(np.float32),
        "vb2": (GAMMA * ASC * b2).reshape(1, DH).astype(np.float32),
        "hwt": np.ascontiguousarray(hw_.T / HSC).astype(np.float16),
        "hb": hb.reshape(DOUT, 1).astype(np.float32),
    }
    in_maps = []
    for c in range(N_CORES):
        shard = x[c * R:(c + 1) * R, :]
        m = dict(shared)
        m["xT"] = np.ascontiguousarray(shard.T).astype(np.float16)
        in_maps.append(m)
    return shared, in_maps


def kernel(**inputs) -> np.ndarray:
    from concourse import bass_utils
    nc = _get_compiled(STEPS)
    _, in_maps = _prep_host(inputs)
    res = None
    for attempt in range(3):
        try:
            res = bass_utils.run_bass_kernel_spmd(
                nc, in_maps, core_ids=list(range(N_CORES)))
            break
        except Exception:
            # transient NRT_EXEC_UNIT_UNRECOVERABLE device wedges clear on
            # retry
            if attempt == 2:
                raise
    out = np.empty((B, DOUT), np.float32)
    for c in range(N_CORES):
        out[c * R:(c + 1) * R, :] = res.results[c]["outT"].T
    return out


if __name__ == "__main__":
    rng = np.random.default_rng(0)
    demo = {
        "x": rng.standard_normal((B, DIN)).astype(np.float32),
        "embed_w": (rng.standard_normal((DH, DIN)) * 0.02).astype(np.float32),
        "embed_b": np.zeros(DH, np.float32),
        "W1": (rng.standard_normal((DFF, DH)) * 0.02).astype(np.float32),
        "b1": np.zeros(DFF, np.float32),
        "W2": (rng.standard_normal((DH, DFF)) * 0.02).astype(np.float32),
        "b2": np.zeros(DH, np.float32),
        "ln_g": np.ones(DH, np.float32),
        "ln_b": np.zeros(DH, np.float32),
        "head_w": (rng.standard_normal((DOUT, DH)) * 0.02).astype(np.float32),
        "head_b": np.zeros(DOUT, np.float32),
    }
    out = kernel(**demo)
    print("out", out.shape, out.dtype, float(np.abs(out).max()))
